# revision 1
# baseline (speedup 1.0000x reference)
"""Trainium2 Bass kernel for 2-layer RGCN (nn_PygModel_52003464020165).

Self-contained: accepts FULL inputs, shards across 8 NeuronCores internally,
returns FULL [64, 10] output.

Architecture (per core, dst-sharded graph):
  - full h replicated each layer via AllGather (bf16, [N, H] row-major in DRAM)
  - per dst-chunk (512 dense dst cols): batched indirect-DMA gather of
    h[src] rows -> msg tiles [128 edges, H] (edges on partitions)
  - per relation r: alpha-hot matrices [128 edges, 128 win] built by one DVE
    tensor_scalar (is_equal vs iota, scaled by 1/cnt); PE matmuls
    msg^T @ alphahot accumulate mean bins into PSUM [H, chunk]
  - transform: root matmul + 20 relation matmuls (W_r stationary, bf16)
    accumulate out^T [H, chunk] in PSUM; evacuation fuses BN partial stats
  - BatchNorm stats via AllReduce; affine+ReLU as one ACT op over [H, NS]
  - PE transposes h^T -> row-major shard -> DRAM -> AllGather
  - global mean pool via indicator matmuls + AllReduce; final linear+sigmoid
"""

import math
import sys

sys.path.insert(0, "/opt/trn_rl_repo")

import ml_dtypes
import numpy as np

import concourse.bacc as bacc
import concourse.bass as bass
import concourse.tile as tile
from concourse import mybir
from concourse import library_config
from concourse.bass_utils import run_bass_kernel_spmd

BF16 = ml_dtypes.bfloat16
P = 128


class Cfg:
    def __init__(self, N=100000, E=1600000, F=64, H=128, R=20, G=64, C=10, L=2,
                 NC=8, CHUNK=512, WIN=128, EPS=1e-5, DT="bf16", DEBUG=False,
                 RSZ=25000):
        assert H == P
        self.N, self.E, self.F, self.H, self.R, self.G, self.C, self.L = (
            N, E, F, H, R, G, C, L)
        self.NC, self.CHUNK, self.WIN, self.EPS = NC, CHUNK, WIN, EPS
        self.DT = DT
        self.DEBUG = DEBUG
        self.RSZ = RSZ
        self.NREG = math.ceil(N / RSZ)
        assert N % NC == 0
        self.NS = N // NC
        self.nchunks = math.ceil(self.NS / CHUNK)
        self.cw = [min(CHUNK, self.NS - c * CHUNK) for c in range(self.nchunks)]
        self.nwin = [math.ceil(w / WIN) for w in self.cw]
        self.SENT = N  # sentinel gather index (> N-1 bound -> skipped)


def _plan(cfg, edge_index, edge_type, batch):
    """Host-side planner. Returns shared structure + per-core data arrays."""
    N, R, NC, NS, CHUNK, WIN = cfg.N, cfg.R, cfg.NC, cfg.NS, cfg.CHUNK, cfg.WIN
    src = edge_index[0].astype(np.int64)
    dst = edge_index[1].astype(np.int64)
    et = edge_type.astype(np.int64)

    comb = dst * R + et
    cnt = np.bincount(comb, minlength=N * R).astype(np.float64)
    alpha_e = (1.0 / np.maximum(cnt[comb], 1.0)).astype(np.float32)

    core = dst // NS
    dloc = dst % NS
    chunk = dloc // CHUNK
    inchunk = dloc % CHUNK
    win = inchunk // WIN
    loc = (inchunk % WIN).astype(np.float32)

    maxwin = max(cfg.nwin)
    gid = (chunk * R + et) * maxwin + win
    ngroups = cfg.nchunks * R * maxwin

    counts = np.zeros((NC, ngroups), np.int64)
    np.add.at(counts, (core, gid), 1)
    Tg = np.maximum(1, -(-counts.max(axis=0) // P))  # ceil div, min 1

    # tile order: chunk-major, then r, then win
    group_order = []
    for c in range(cfg.nchunks):
        for r in range(R):
            for w in range(cfg.nwin[c]):
                group_order.append((c * R + r) * maxwin + w)
    group_order = np.array(group_order, np.int64)
    tiles_of_group = Tg[group_order]
    tile_base = np.zeros(len(group_order), np.int64)
    np.cumsum(tiles_of_group[:-1], out=tile_base[1:])
    S_total = int(tiles_of_group.sum())

    gpos = np.full(ngroups, -1, np.int64)
    gpos[group_order] = np.arange(len(group_order))

    locA = np.full((NC, P, S_total), -1.0, np.float32)
    alphaA = np.zeros((NC, P, S_total), np.float32)
    srcA = np.zeros((NC, P, S_total), np.int64)  # src per slot (sentinel: -1)
    srcA[:] = -1

    order = np.lexsort((gid, core))
    s_core, s_gid = core[order], gid[order]
    s_src, s_loc, s_alpha = src[order], loc[order], alpha_e[order]
    key = s_core * ngroups + s_gid
    first = np.r_[True, key[1:] != key[:-1]]
    grp_start = np.flatnonzero(first)
    seglen = np.diff(np.r_[grp_start, len(key)])
    rank = np.arange(len(key)) - np.repeat(grp_start, seglen)

    slot = tile_base[gpos[s_gid]] * P + rank
    srcA[s_core, slot % P, slot // P] = s_src
    locA[s_core, slot % P, slot // P] = s_loc
    alphaA[s_core, slot % P, slot // P] = s_alpha

    # emission structure: per chunk -> (slot_lo, slot_hi,
    #   per-r list of per-win (tile_base, ntiles))
    chunk_tiles = []
    for c in range(cfg.nchunks):
        lo = None
        hi = 0
        rlists = []
        for r in range(R):
            wl = []
            for w in range(cfg.nwin[c]):
                pos = gpos[(c * R + r) * maxwin + w]
                tb, tn = int(tile_base[pos]), int(tiles_of_group[pos])
                if lo is None:
                    lo = tb
                hi = tb + tn
                wl.append((tb, tn, w))
            rlists.append(wl)
        chunk_tiles.append((lo, hi, rlists))

    idxA = np.where(srcA >= 0, srcA, 0).astype(np.int32)

    gcnt = np.bincount(batch.astype(np.int64), minlength=cfg.G).astype(np.float32)
    inv_gcnt = 1.0 / np.maximum(gcnt, 1.0)

    return dict(S_total=S_total, chunk_tiles=chunk_tiles, locA=locA,
                alphaA=alphaA, inv_gcnt=inv_gcnt, idxA=idxA)


def _build_nc(cfg, plan):
    """Emit the SPMD Bass program (one program, NC cores)."""
    N, F, H, R, G, C, L = cfg.N, cfg.F, cfg.H, cfg.R, cfg.G, cfg.C, cfg.L
    NS, CHUNK, WIN = cfg.NS, cfg.CHUNK, cfg.WIN
    S_total = plan["S_total"]
    chunk_tiles = plan["chunk_tiles"]
    nblk = math.ceil(NS / P)

    nc = bacc.Bacc(None)
    f32, i32, i16 = mybir.dt.float32, mybir.dt.int32, mybir.dt.int16
    bf16 = mybir.dt.bfloat16 if cfg.DT == "bf16" else mybir.dt.float32
    AF = mybir.ActivationFunctionType
    OP = mybir.AluOpType

    xT_d = nc.dram_tensor("xT", [F, NS], f32, kind="ExternalInput")
    idx_d = nc.dram_tensor("idxA", [P, S_total], i32, kind="ExternalInput")
    iota_d = nc.dram_tensor("iotain", [P, WIN], bf16, kind="ExternalInput")
    identb_d = nc.dram_tensor("identb", [P, P], bf16, kind="ExternalInput")
    identf_d = nc.dram_tensor("identf", [P, P], f32, kind="ExternalInput")
    loc_d = nc.dram_tensor("locA", [P, S_total], f32, kind="ExternalInput")
    alp_d = nc.dram_tensor("alphaA", [P, S_total], f32, kind="ExternalInput")
    w_in_d = nc.dram_tensor("w_in", [F, H], f32, kind="ExternalInput")
    b_in_d = nc.dram_tensor("b_in", [H, 1], f32, kind="ExternalInput")
    relw_d = nc.dram_tensor("relw", [P, L * R + L, H], bf16, kind="ExternalInput")
    bng_d = nc.dram_tensor("bng", [H, L], f32, kind="ExternalInput")
    bnb_d = nc.dram_tensor("bnb", [H, L], f32, kind="ExternalInput")
    w_out_d = nc.dram_tensor("w_out", [H, C], f32, kind="ExternalInput")
    b_out_d = nc.dram_tensor("b_out", [C, 1], f32, kind="ExternalInput")
    gids_d = nc.dram_tensor("gids", [P, nblk], f32, kind="ExternalInput")
    invg_d = nc.dram_tensor("invg", [C, G], f32, kind="ExternalInput")
    out_d = nc.dram_tensor("out", [C, G], f32, kind="ExternalOutput")

    h_shard = [nc.dram_tensor(f"h_shard{l}", [NS, H], bf16) for l in range(L)]
    h_full = [nc.dram_tensor(f"h_full{l}", [N, H], bf16, addr_space="Shared")
              for l in range(L)]
    stats_in = nc.dram_tensor("stats_in", [H, 2], f32)
    stats_out = nc.dram_tensor("stats_out", [H, 2], f32, addr_space="Shared")
    pool_in = nc.dram_tensor("pool_in", [G, H], f32)
    pool_out = nc.dram_tensor("pool_out", [G, H], f32, addr_space="Shared")
    if cfg.DEBUG:
        dbg_h = [nc.dram_tensor(f"dbg_h{l}", [N, H], bf16,
                                kind="ExternalOutput") for l in range(L)]
        dbg_outb = nc.dram_tensor("dbg_outb", [L, H, NS], bf16,
                                  kind="ExternalOutput")
        dbg_stg = nc.dram_tensor("dbg_stg", [L, H, 8], f32,
                                 kind="ExternalOutput")
        dbg_pool = nc.dram_tensor("dbg_pool", [G, H], f32,
                                  kind="ExternalOutput")

    cores = list(range(cfg.NC))

    with tile.TileContext(nc) as tc:
        with (
            tc.tile_pool(name="const", bufs=1) as cpool,
            tc.tile_pool(name="big", bufs=1) as bigpool,
            tc.tile_pool(name="msg", bufs=2) as msgpool,
            tc.tile_pool(name="hot", bufs=16) as hotpool,
            tc.tile_pool(name="mean", bufs=2) as meanpool,
            tc.tile_pool(name="work", bufs=3) as workpool,
            tc.tile_pool(name="psA", bufs=2, space="PSUM") as psA,
            tc.tile_pool(name="psT", bufs=2, space="PSUM") as psT,
            tc.tile_pool(name="psB", bufs=2, space="PSUM") as psB,
        ):
            # ---------- constants ----------
            iota_bf = cpool.tile([P, WIN], bf16, tag="iota_bf")
            nc.sync.dma_start(iota_bf[:], iota_d[:])
            ident = cpool.tile([P, P], bf16, tag="ident")
            nc.sync.dma_start(ident[:], identb_d[:])
            identf = cpool.tile([P, P], f32, tag="identf")
            nc.sync.dma_start(identf[:], identf_d[:])

            idx_t = cpool.tile([P, S_total], i32, tag="idx")
            nc.sync.dma_start(idx_t[:], idx_d[:])
            loc_t = cpool.tile([P, S_total], f32, tag="loc")
            nc.sync.dma_start(loc_t[:], loc_d[:])
            alp_t = cpool.tile([P, S_total], f32, tag="alp")
            nc.sync.dma_start(alp_t[:], alp_d[:])

            relw_t = cpool.tile([P, L * R + L, H], bf16, tag="relw")
            nc.sync.dma_start(relw_t[:], relw_d[:])
            w_in_t = cpool.tile([F, H], f32, tag="w_in")
            nc.sync.dma_start(w_in_t[:], w_in_d[:])
            b_in_t = cpool.tile([H, 1], f32, tag="b_in")
            nc.sync.dma_start(b_in_t[:], b_in_d[:])
            bng_t = cpool.tile([H, L], f32, tag="bng")
            nc.sync.dma_start(bng_t[:], bng_d[:])
            bnb_t = cpool.tile([H, L], f32, tag="bnb")
            nc.sync.dma_start(bnb_t[:], bnb_d[:])
            gids_t = cpool.tile([P, nblk], f32, tag="gids")
            nc.sync.dma_start(gids_t[:], gids_d[:])
            w_out_t = cpool.tile([H, C], f32, tag="w_out")
            nc.sync.dma_start(w_out_t[:], w_out_d[:])
            b_out_t = cpool.tile([C, 1], f32, tag="b_out")
            nc.sync.dma_start(b_out_t[:], b_out_d[:])
            invg_t = cpool.tile([C, G], f32, tag="invg")
            nc.sync.dma_start(invg_t[:], invg_d[:])

            # fences: pull const-load DMA completions into engine program
            # order one DMA at a time, so compute ops (tiny ISA wait
            # budgets) emit no DMA waits of their own
            fence = cpool.tile([1, 1], f32, tag="fence")
            for _ft in (loc_t, alp_t, gids_t, bng_t, bnb_t, invg_t, w_in_t,
                        w_out_t):
                nc.vector.tensor_copy(fence[:], _ft[0:1, 0:1].bitcast(f32))
            fenceA = cpool.tile([1, 1], f32, tag="fenceA")
            for _ft in (b_in_t, b_out_t):
                nc.scalar.copy(fenceA[:], _ft[0:1, 0:1])
            _rw = 2 if cfg.DT == "bf16" else 1
            nc.scalar.copy(fenceA[:], relw_t[0:1, 0, 0:_rw].bitcast(f32))

            hT = bigpool.tile([P, NS], bf16, tag="hT")
            outb = bigpool.tile([P, NS], bf16, tag="outb")
            sum_parts = bigpool.tile([P, cfg.nchunks], f32, tag="sumP")
            sq_parts = bigpool.tile([P, cfg.nchunks], f32, tag="sqP")
            sq_scr = bigpool.tile([P, CHUNK], bf16, tag="sqscr")

            # ---------- input MLP ----------
            for c in range(cfg.nchunks):
                cw = cfg.cw[c]
                xc = workpool.tile([F, CHUNK], f32, tag="xc")
                nc.sync.dma_start(xc[:, :cw], xT_d[:, c * CHUNK:c * CHUNK + cw])
                ps = psB.tile([P, CHUNK], f32, tag="psB")
                nc.tensor.matmul(out=ps[:, :cw], lhsT=w_in_t[:], rhs=xc[:, :cw],
                                 start=True, stop=True)
                nc.scalar.activation(hT[:, c * CHUNK:c * CHUNK + cw], ps[:, :cw],
                                     AF.Relu, bias=b_in_t[:, 0:1], scale=1.0)

            def emit_transpose_store(l):
                for b in range(nblk):
                    bw = min(P, NS - b * P)
                    pst = psT.tile([P, P], bf16, tag="psT")
                    nc.tensor.transpose(pst[:bw, :P], hT[:, b * P:b * P + bw],
                                        ident[:])
                    rm = workpool.tile([P, P], bf16, tag="rm")
                    nc.vector.tensor_copy(rm[:bw, :], pst[:bw, :P])
                    nc.sync.dma_start(h_shard[l][b * P:b * P + bw, :], rm[:bw, :])
                nc.gpsimd.collective_compute(
                    "AllGather", OP.bypass, replica_groups=[cores],
                    ins=[h_shard[l][:]], outs=[h_full[l][:]])
                if cfg.DEBUG:
                    nc.gpsimd.dma_start(dbg_h[l][:], h_full[l][:])

            emit_transpose_store(0)

            # ---------- RGCN layers ----------
            for l in range(L):
                root_i = L * R + l
                for c in range(cfg.nchunks):
                    cw = cfg.cw[c]
                    lo, hi, rlists = chunk_tiles[c]
                    nS = hi - lo
                    msg = msgpool.tile([P, nS, H], bf16, tag="msg")
                    for s in range(nS):
                        nc.gpsimd.indirect_dma_start(
                            out=msg[:, s, :], out_offset=None,
                            in_=h_full[l][:],
                            in_offset=bass.IndirectOffsetOnAxis(
                                ap=idx_t[:, lo + s:lo + s + 1], axis=0))

                    mean = meanpool.tile([P, R, CHUNK], bf16, tag="mean")
                    for r in range(R):
                        psa = psA.tile([P, CHUNK], f32, tag="psA")
                        for (tb, tn, w) in rlists[r]:
                            ww = min(WIN, cw - w * WIN)
                            for t in range(tn):
                                s = tb + t
                                hot = hotpool.tile([P, WIN], bf16, tag="hot")
                                nc.vector.tensor_scalar(
                                    out=hot[:, :ww], in0=iota_bf[:, :ww],
                                    scalar1=loc_t[:, s:s + 1],
                                    scalar2=alp_t[:, s:s + 1],
                                    op0=OP.is_equal, op1=OP.mult)
                                nc.tensor.matmul(
                                    out=psa[:, w * WIN:w * WIN + ww],
                                    lhsT=msg[:, s - lo, :], rhs=hot[:, :ww],
                                    start=(t == 0), stop=(t == tn - 1))
                        if r % 2 == 0:
                            nc.vector.tensor_copy(mean[:, r, :cw], psa[:, :cw])
                        else:
                            nc.scalar.copy(mean[:, r, :cw], psa[:, :cw])

                    psb = psB.tile([P, CHUNK], f32, tag="psB")
                    nc.tensor.matmul(out=psb[:, :cw], lhsT=relw_t[:, root_i, :],
                                     rhs=hT[:, c * CHUNK:c * CHUNK + cw],
                                     start=True, stop=False)
                    for r in range(R):
                        nc.tensor.matmul(out=psb[:, :cw],
                                         lhsT=relw_t[:, l * R + r, :],
                                         rhs=mean[:, r, :cw],
                                         start=False, stop=(r == R - 1))

                    nc.vector.tensor_scalar(
                        out=outb[:, c * CHUNK:c * CHUNK + cw], in0=psb[:, :cw],
                        scalar1=1.0, scalar2=None, op0=OP.mult, op1=OP.add,
                        accum_out=sum_parts[:, c:c + 1])
                    nc.scalar.activation(sq_scr[:, :cw], psb[:, :cw], AF.Square,
                                         accum_out=sq_parts[:, c:c + 1])

                # ---------- BatchNorm + ReLU ----------
                st = workpool.tile([H, 2], f32, tag="stats")
                nc.vector.reduce_sum(st[:, 0:1], sum_parts[:],
                                     axis=mybir.AxisListType.X)
                nc.vector.reduce_sum(st[:, 1:2], sq_parts[:],
                                     axis=mybir.AxisListType.X)
                nc.sync.dma_start(stats_in[:], st[:])
                nc.gpsimd.collective_compute(
                    "AllReduce", OP.add, replica_groups=[cores],
                    ins=[stats_in[:]], outs=[stats_out[:]])
                stg = workpool.tile([H, 8], f32, tag="stg")
                nc.sync.dma_start(stg[:, 0:2], stats_out[:])
                nc.vector.tensor_scalar(out=stg[:, 2:3], in0=stg[:, 0:1],
                                        scalar1=1.0 / N, scalar2=None,
                                        op0=OP.mult)
                nc.vector.tensor_scalar(out=stg[:, 3:4], in0=stg[:, 1:2],
                                        scalar1=1.0 / N, scalar2=None,
                                        op0=OP.mult)
                nc.vector.tensor_tensor(out=stg[:, 4:5], in0=stg[:, 2:3],
                                        in1=stg[:, 2:3], op=OP.mult)
                nc.vector.tensor_tensor(out=stg[:, 4:5], in0=stg[:, 3:4],
                                        in1=stg[:, 4:5], op=OP.subtract)
                nc.vector.tensor_scalar(out=stg[:, 4:5], in0=stg[:, 4:5],
                                        scalar1=cfg.EPS, scalar2=None,
                                        op0=OP.add)
                nc.scalar.sqrt(stg[:, 5:6], stg[:, 4:5])
                nc.vector.reciprocal(stg[:, 6:7], stg[:, 5:6])
                nc.vector.tensor_tensor(out=stg[:, 6:7], in0=stg[:, 6:7],
                                        in1=bng_t[:, l:l + 1], op=OP.mult)
                nc.vector.tensor_tensor(out=stg[:, 7:8], in0=stg[:, 6:7],
                                        in1=stg[:, 2:3], op=OP.mult)
                nc.vector.tensor_tensor(out=stg[:, 7:8], in0=bnb_t[:, l:l + 1],
                                        in1=stg[:, 7:8], op=OP.subtract)
                if cfg.DEBUG:
                    nc.sync.dma_start(dbg_outb[l], outb[:])
                    nc.sync.dma_start(dbg_stg[l], stg[:])
                nc.scalar.activation(hT[:], outb[:], AF.Relu,
                                     bias=stg[:, 7:8], scale=stg[:, 6:7])

                if l + 1 < L:
                    emit_transpose_store(l + 1)

            # ---------- global mean pool + output MLP ----------
            psp = psB.tile([G, CHUNK], f32, tag="psB")
            for b in range(nblk):
                bw = min(P, NS - b * P)
                pst = psT.tile([P, P], bf16, tag="psT")
                nc.tensor.transpose(pst[:bw, :P], hT[:, b * P:b * P + bw],
                                    ident[:])
                rm = workpool.tile([P, P], bf16, tag="rm")
                nc.vector.tensor_copy(rm[:bw, :], pst[:bw, :P])
                ind = hotpool.tile([P, G], bf16, tag="ind")
                nc.vector.tensor_scalar(out=ind[:bw, :], in0=iota_bf[:bw, :G],
                                        scalar1=gids_t[:bw, b:b + 1],
                                        scalar2=None, op0=OP.is_equal)
                nc.tensor.matmul(out=psp[:, :H], lhsT=ind[:bw, :],
                                 rhs=rm[:bw, :], start=(b == 0),
                                 stop=(b == nblk - 1))
            poolt = workpool.tile([G, H], f32, tag="poolt")
            nc.vector.tensor_copy(poolt[:], psp[:, :H])
            nc.sync.dma_start(pool_in[:], poolt[:])
            nc.gpsimd.collective_compute(
                "AllReduce", OP.add, replica_groups=[cores],
                ins=[pool_in[:]], outs=[pool_out[:]])
            poolg = workpool.tile([G, H], f32, tag="poolg")
            nc.sync.dma_start(poolg[:], pool_out[:])
            if cfg.DEBUG:
                nc.gpsimd.dma_start(dbg_pool[:], pool_out[:])

            pstT = psT.tile([P, G], f32, tag="psTf")
            nc.tensor.transpose(pstT[:, :G], poolg[:], identf[:G, :G])
            poolT = workpool.tile([P, G], f32, tag="poolT")
            nc.vector.tensor_copy(poolT[:], pstT[:, :G])

            psl = psB.tile([C, CHUNK], f32, tag="psB")
            nc.tensor.matmul(out=psl[:, :G], lhsT=w_out_t[:], rhs=poolT[:],
                             start=True, stop=True)
            logit = workpool.tile([C, G], f32, tag="logit")
            nc.vector.tensor_tensor(out=logit[:], in0=psl[:, :G], in1=invg_t[:],
                                    op=OP.mult)
            logit2 = workpool.tile([C, G], f32, tag="logit2")
            nc.scalar.activation(logit2[:], logit[:], AF.Sigmoid,
                                 bias=b_out_t[:, 0:1], scale=1.0)
            nc.sync.dma_start(out_d[:], logit2[:])

    return nc


def _make_in_maps(cfg, plan, inputs):
    H, C, G, F, NS, R, L = cfg.H, cfg.C, cfg.G, cfg.F, cfg.NS, cfg.R, cfg.L
    x = np.asarray(inputs["x"], np.float32)
    batch = np.asarray(inputs["batch"])

    np_dt = BF16 if cfg.DT == "bf16" else np.float32
    relw = np.empty((L * R + L, P, H), np_dt)
    rel_w = np.asarray(inputs["rel_w"], np.float32)
    root_w = np.asarray(inputs["root_w"], np.float32)
    for l in range(L):
        for r in range(R):
            relw[l * R + r] = rel_w[l, r].astype(np_dt)
        relw[L * R + l] = root_w[l].astype(np_dt)

    bng = np.ascontiguousarray(np.asarray(inputs["bn_g"], np.float32).T)
    bnb = np.ascontiguousarray(np.asarray(inputs["bn_b"], np.float32).T)
    b_in = np.asarray(inputs["b_in"], np.float32).reshape(H, 1)
    b_out = np.asarray(inputs["b_out"], np.float32).reshape(C, 1)
    w_in = np.asarray(inputs["w_in"], np.float32)
    w_out = np.asarray(inputs["w_out"], np.float32)
    invg = np.ascontiguousarray(
        np.broadcast_to(plan["inv_gcnt"][None, :], (C, G)))
    relw_p = np.ascontiguousarray(relw.transpose(1, 0, 2))  # [P, 42, H]
    iota_np = np.broadcast_to(np.arange(cfg.WIN, dtype=np.float32)[None, :],
                              (P, cfg.WIN)).astype(BF16).copy()
    ident_np = np.eye(P, dtype=np.float32).astype(BF16)
    identf_np = np.eye(P, dtype=np.float32)

    nblk = math.ceil(NS / P)
    in_maps = []
    for c in range(cfg.NC):
        lo, hi = c * NS, (c + 1) * NS
        xT = np.ascontiguousarray(x[lo:hi].T)
        gids = np.full((P, nblk), -1.0, np.float32)
        bseg = batch[lo:hi].astype(np.float32)
        for b in range(nblk):
            bw = min(P, NS - b * P)
            gids[:bw, b] = bseg[b * P:b * P + bw]
        in_maps.append(dict(
            xT=xT, idxA=np.ascontiguousarray(plan["idxA"][c]),
            iotain=iota_np, identb=ident_np, identf=identf_np,
            locA=np.ascontiguousarray(plan["locA"][c]),
            alphaA=np.ascontiguousarray(plan["alphaA"][c]),
            w_in=w_in, b_in=b_in, relw=relw_p, bng=bng, bnb=bnb,
            w_out=w_out, b_out=b_out, gids=gids, invg=invg,
        ))
    return in_maps


def _run(cfg, inputs, **kw):
    plan = _plan(cfg, np.asarray(inputs["edge_index"]),
                 np.asarray(inputs["edge_type"]), np.asarray(inputs["batch"]))
    nc = _build_nc(cfg, plan)
    if not nc.is_finalized():
        nc.finalize()
    in_maps = _make_in_maps(cfg, plan, inputs)
    res = run_bass_kernel_spmd(nc, in_maps, core_ids=list(range(cfg.NC)), **kw)
    out = res.results[0]["out"]
    return np.ascontiguousarray(np.asarray(out).T.astype(np.float32)), res


def kernel(**inputs):
    cfg = Cfg()
    out, _ = _run(cfg, inputs)
    return out



# revision 2
# speedup vs baseline: 242.1703x; 242.1703x over previous
"""Trainium2 Bass kernel for 2-layer RGCN (nn_PygModel_52003464020165).

Self-contained: accepts FULL inputs, shards across 8 NeuronCores internally,
returns FULL [64, 10] output.

Architecture (per core, dst-sharded graph):
  - full h replicated each layer via AllGather (bf16, [N, H] row-major in DRAM)
  - per dst-chunk (512 dense dst cols): batched indirect-DMA gather of
    h[src] rows -> msg tiles [128 edges, H] (edges on partitions)
  - per relation r: alpha-hot matrices [128 edges, 128 win] built by one DVE
    tensor_scalar (is_equal vs iota, scaled by 1/cnt); PE matmuls
    msg^T @ alphahot accumulate mean bins into PSUM [H, chunk]
  - transform: root matmul + 20 relation matmuls (W_r stationary, bf16)
    accumulate out^T [H, chunk] in PSUM; evacuation fuses BN partial stats
  - BatchNorm stats via AllReduce; affine+ReLU as one ACT op over [H, NS]
  - PE transposes h^T -> row-major shard -> DRAM -> AllGather
  - global mean pool via indicator matmuls + AllReduce; final linear+sigmoid
"""

import math
import sys

sys.path.insert(0, "/opt/trn_rl_repo")

import ml_dtypes
import numpy as np

import concourse.bacc as bacc
import concourse.bass as bass
import concourse.tile as tile
from concourse import mybir
from concourse import library_config
from concourse.bass_utils import run_bass_kernel_spmd

BF16 = ml_dtypes.bfloat16
P = 128


class Cfg:
    def __init__(self, N=100000, E=1600000, F=64, H=128, R=20, G=64, C=10, L=2,
                 NC=8, CHUNK=512, WIN=128, EPS=1e-5, DT="bf16", DEBUG=False,
                 RSZ=25000):
        assert H == P
        self.N, self.E, self.F, self.H, self.R, self.G, self.C, self.L = (
            N, E, F, H, R, G, C, L)
        self.NC, self.CHUNK, self.WIN, self.EPS = NC, CHUNK, WIN, EPS
        self.DT = DT
        self.DEBUG = DEBUG
        self.RSZ = RSZ
        self.NREG = math.ceil(N / RSZ)
        assert N % NC == 0
        self.NS = N // NC
        self.nchunks = math.ceil(self.NS / CHUNK)
        self.cw = [min(CHUNK, self.NS - c * CHUNK) for c in range(self.nchunks)]
        self.nwin = [math.ceil(w / WIN) for w in self.cw]
        self.SENT = N  # sentinel gather index (> N-1 bound -> skipped)


def _plan(cfg, edge_index, edge_type, batch):
    """Host-side planner. Returns shared structure + per-core data arrays."""
    N, R, NC, NS, CHUNK, WIN = cfg.N, cfg.R, cfg.NC, cfg.NS, cfg.CHUNK, cfg.WIN
    src = edge_index[0].astype(np.int64)
    dst = edge_index[1].astype(np.int64)
    et = edge_type.astype(np.int64)

    comb = dst * R + et
    cnt = np.bincount(comb, minlength=N * R).astype(np.float64)
    alpha_e = (1.0 / np.maximum(cnt[comb], 1.0)).astype(np.float32)

    core = dst // NS
    dloc = dst % NS
    chunk = dloc // CHUNK
    inchunk = dloc % CHUNK
    win = inchunk // WIN
    loc = (inchunk % WIN).astype(np.float32)

    maxwin = max(cfg.nwin)
    gid = (chunk * R + et) * maxwin + win
    ngroups = cfg.nchunks * R * maxwin

    counts = np.zeros((NC, ngroups), np.int64)
    np.add.at(counts, (core, gid), 1)
    Tg = np.maximum(1, -(-counts.max(axis=0) // P))  # ceil div, min 1

    # tile order: chunk-major, then r, then win
    group_order = []
    for c in range(cfg.nchunks):
        for r in range(R):
            for w in range(cfg.nwin[c]):
                group_order.append((c * R + r) * maxwin + w)
    group_order = np.array(group_order, np.int64)
    tiles_of_group = Tg[group_order]
    tile_base = np.zeros(len(group_order), np.int64)
    np.cumsum(tiles_of_group[:-1], out=tile_base[1:])
    S_total = int(tiles_of_group.sum())

    gpos = np.full(ngroups, -1, np.int64)
    gpos[group_order] = np.arange(len(group_order))

    locA = np.full((NC, P, S_total), -1.0, np.float32)
    alphaA = np.zeros((NC, P, S_total), np.float32)
    srcA = np.zeros((NC, P, S_total), np.int64)  # src per slot (sentinel: -1)
    srcA[:] = -1

    order = np.lexsort((gid, core))
    s_core, s_gid = core[order], gid[order]
    s_src, s_loc, s_alpha = src[order], loc[order], alpha_e[order]
    key = s_core * ngroups + s_gid
    first = np.r_[True, key[1:] != key[:-1]]
    grp_start = np.flatnonzero(first)
    seglen = np.diff(np.r_[grp_start, len(key)])
    rank = np.arange(len(key)) - np.repeat(grp_start, seglen)

    slot = tile_base[gpos[s_gid]] * P + rank
    srcA[s_core, slot % P, slot // P] = s_src
    locA[s_core, slot % P, slot // P] = s_loc
    alphaA[s_core, slot % P, slot // P] = s_alpha

    # emission structure: per chunk -> (slot_lo, slot_hi,
    #   per-r list of per-win (tile_base, ntiles))
    chunk_tiles = []
    for c in range(cfg.nchunks):
        lo = None
        hi = 0
        rlists = []
        for r in range(R):
            wl = []
            for w in range(cfg.nwin[c]):
                pos = gpos[(c * R + r) * maxwin + w]
                tb, tn = int(tile_base[pos]), int(tiles_of_group[pos])
                if lo is None:
                    lo = tb
                hi = tb + tn
                wl.append((tb, tn, w))
            rlists.append(wl)
        chunk_tiles.append((lo, hi, rlists))

    idxA = np.where(srcA >= 0, srcA, 0).astype(np.int32)

    gcnt = np.bincount(batch.astype(np.int64), minlength=cfg.G).astype(np.float32)
    inv_gcnt = 1.0 / np.maximum(gcnt, 1.0)

    return dict(S_total=S_total, chunk_tiles=chunk_tiles, locA=locA,
                alphaA=alphaA, inv_gcnt=inv_gcnt, idxA=idxA)


def _build_nc(cfg, plan):
    """Emit the SPMD Bass program (one program, NC cores)."""
    N, F, H, R, G, C, L = cfg.N, cfg.F, cfg.H, cfg.R, cfg.G, cfg.C, cfg.L
    NS, CHUNK, WIN = cfg.NS, cfg.CHUNK, cfg.WIN
    S_total = plan["S_total"]
    chunk_tiles = plan["chunk_tiles"]
    nblk = math.ceil(NS / P)

    nc = bacc.Bacc(None)
    f32, i32, i16 = mybir.dt.float32, mybir.dt.int32, mybir.dt.int16
    bf16 = mybir.dt.bfloat16 if cfg.DT == "bf16" else mybir.dt.float32
    AF = mybir.ActivationFunctionType
    OP = mybir.AluOpType

    xT_d = nc.dram_tensor("xT", [F, NS], f32, kind="ExternalInput")
    idx_d = nc.dram_tensor("idxA", [P, S_total], i32, kind="ExternalInput")
    iota_d = nc.dram_tensor("iotain", [P, WIN], bf16, kind="ExternalInput")
    identb_d = nc.dram_tensor("identb", [P, P], bf16, kind="ExternalInput")
    identf_d = nc.dram_tensor("identf", [P, P], f32, kind="ExternalInput")
    loc_d = nc.dram_tensor("locA", [P, S_total], f32, kind="ExternalInput")
    alp_d = nc.dram_tensor("alphaA", [P, S_total], f32, kind="ExternalInput")
    w_in_d = nc.dram_tensor("w_in", [F, H], f32, kind="ExternalInput")
    b_in_d = nc.dram_tensor("b_in", [H, 1], f32, kind="ExternalInput")
    relw_d = nc.dram_tensor("relw", [P, L * R + L, H], bf16, kind="ExternalInput")
    bng_d = nc.dram_tensor("bng", [H, L], f32, kind="ExternalInput")
    bnb_d = nc.dram_tensor("bnb", [H, L], f32, kind="ExternalInput")
    w_out_d = nc.dram_tensor("w_out", [H, C], f32, kind="ExternalInput")
    b_out_d = nc.dram_tensor("b_out", [C, 1], f32, kind="ExternalInput")
    gids_d = nc.dram_tensor("gids", [P, nblk], f32, kind="ExternalInput")
    invg_d = nc.dram_tensor("invg", [C, G], f32, kind="ExternalInput")
    out_d = nc.dram_tensor("out", [C, G], f32, kind="ExternalOutput")

    h_shard = [nc.dram_tensor(f"h_shard{l}", [NS, H], bf16) for l in range(L)]
    h_full = [nc.dram_tensor(f"h_full{l}", [N, H], bf16, addr_space="Shared")
              for l in range(L)]
    stats_in = nc.dram_tensor("stats_in", [H, 2], f32)
    stats_out = nc.dram_tensor("stats_out", [H, 2], f32, addr_space="Shared")
    pool_in = nc.dram_tensor("pool_in", [G, H], f32)
    pool_out = nc.dram_tensor("pool_out", [G, H], f32, addr_space="Shared")
    if cfg.DEBUG:
        dbg_h = [nc.dram_tensor(f"dbg_h{l}", [N, H], bf16,
                                kind="ExternalOutput") for l in range(L)]
        dbg_outb = nc.dram_tensor("dbg_outb", [L, H, NS], bf16,
                                  kind="ExternalOutput")
        dbg_stg = nc.dram_tensor("dbg_stg", [L, H, 8], f32,
                                 kind="ExternalOutput")
        dbg_pool = nc.dram_tensor("dbg_pool", [G, H], f32,
                                  kind="ExternalOutput")

    cores = list(range(cfg.NC))

    with tile.TileContext(nc) as tc:
        with (
            tc.tile_pool(name="const", bufs=1) as cpool,
            tc.tile_pool(name="big", bufs=1) as bigpool,
            tc.tile_pool(name="msg", bufs=2) as msgpool,
            tc.tile_pool(name="hot", bufs=16) as hotpool,
            tc.tile_pool(name="mean", bufs=2) as meanpool,
            tc.tile_pool(name="work", bufs=3) as workpool,
            tc.tile_pool(name="psA", bufs=2, space="PSUM") as psA,
            tc.tile_pool(name="psT", bufs=2, space="PSUM") as psT,
            tc.tile_pool(name="psB", bufs=2, space="PSUM") as psB,
        ):
            # ---------- constants ----------
            iota_bf = cpool.tile([P, WIN], bf16, tag="iota_bf")
            nc.sync.dma_start(iota_bf[:], iota_d[:])
            ident = cpool.tile([P, P], bf16, tag="ident")
            nc.sync.dma_start(ident[:], identb_d[:])
            identf = cpool.tile([P, P], f32, tag="identf")
            nc.sync.dma_start(identf[:], identf_d[:])

            idx_t = cpool.tile([P, S_total], i32, tag="idx")
            nc.sync.dma_start(idx_t[:], idx_d[:])
            loc_t = cpool.tile([P, S_total], f32, tag="loc")
            nc.sync.dma_start(loc_t[:], loc_d[:])
            alp_t = cpool.tile([P, S_total], f32, tag="alp")
            nc.sync.dma_start(alp_t[:], alp_d[:])

            relw_t = cpool.tile([P, L * R + L, H], bf16, tag="relw")
            nc.sync.dma_start(relw_t[:], relw_d[:])
            w_in_t = cpool.tile([F, H], f32, tag="w_in")
            nc.sync.dma_start(w_in_t[:], w_in_d[:])
            b_in_t = cpool.tile([H, 1], f32, tag="b_in")
            nc.sync.dma_start(b_in_t[:], b_in_d[:])
            bng_t = cpool.tile([H, L], f32, tag="bng")
            nc.sync.dma_start(bng_t[:], bng_d[:])
            bnb_t = cpool.tile([H, L], f32, tag="bnb")
            nc.sync.dma_start(bnb_t[:], bnb_d[:])
            gids_t = cpool.tile([P, nblk], f32, tag="gids")
            nc.sync.dma_start(gids_t[:], gids_d[:])
            w_out_t = cpool.tile([H, C], f32, tag="w_out")
            nc.sync.dma_start(w_out_t[:], w_out_d[:])
            b_out_t = cpool.tile([C, 1], f32, tag="b_out")
            nc.sync.dma_start(b_out_t[:], b_out_d[:])
            invg_t = cpool.tile([C, G], f32, tag="invg")
            nc.sync.dma_start(invg_t[:], invg_d[:])

            # fences: pull const-load DMA completions into engine program
            # order one DMA at a time, so compute ops (tiny ISA wait
            # budgets) emit no DMA waits of their own
            fence = cpool.tile([1, 1], f32, tag="fence")
            for _ft in (loc_t, alp_t, gids_t, bng_t, bnb_t, invg_t, w_in_t,
                        w_out_t):
                nc.vector.tensor_copy(fence[:], _ft[0:1, 0:1].bitcast(f32))
            fenceA = cpool.tile([1, 1], f32, tag="fenceA")
            for _ft in (b_in_t, b_out_t):
                nc.scalar.copy(fenceA[:], _ft[0:1, 0:1])
            _rw = 2 if cfg.DT == "bf16" else 1
            nc.scalar.copy(fenceA[:], relw_t[0:1, 0, 0:_rw].bitcast(f32))

            hT = bigpool.tile([P, NS], bf16, tag="hT")
            outb = bigpool.tile([P, NS], bf16, tag="outb")
            sum_parts = bigpool.tile([P, cfg.nchunks], f32, tag="sumP")
            sq_parts = bigpool.tile([P, cfg.nchunks], f32, tag="sqP")
            sq_scr = bigpool.tile([P, CHUNK], bf16, tag="sqscr")

            # ---------- input MLP ----------
            for c in range(cfg.nchunks):
                cw = cfg.cw[c]
                xc = workpool.tile([F, CHUNK], f32, tag="xc")
                nc.sync.dma_start(xc[:, :cw], xT_d[:, c * CHUNK:c * CHUNK + cw])
                ps = psB.tile([P, CHUNK], f32, tag="psB")
                nc.tensor.matmul(out=ps[:, :cw], lhsT=w_in_t[:], rhs=xc[:, :cw],
                                 start=True, stop=True)
                nc.scalar.activation(hT[:, c * CHUNK:c * CHUNK + cw], ps[:, :cw],
                                     AF.Relu, bias=b_in_t[:, 0:1], scale=1.0)

            def emit_transpose_store(l):
                for b in range(nblk):
                    bw = min(P, NS - b * P)
                    pst = psT.tile([P, P], bf16, tag="psT")
                    nc.tensor.transpose(pst[:bw, :P], hT[:, b * P:b * P + bw],
                                        ident[:])
                    rm = workpool.tile([P, P], bf16, tag="rm")
                    nc.vector.tensor_copy(rm[:bw, :], pst[:bw, :P])
                    nc.sync.dma_start(h_shard[l][b * P:b * P + bw, :], rm[:bw, :])
                nc.gpsimd.collective_compute(
                    "AllGather", OP.bypass, replica_groups=[cores],
                    ins=[h_shard[l][:]], outs=[h_full[l][:]])
                if cfg.DEBUG:
                    nc.gpsimd.dma_start(dbg_h[l][:], h_full[l][:])

            emit_transpose_store(0)

            # ---------- RGCN layers ----------
            for l in range(L):
                root_i = L * R + l
                for c in range(cfg.nchunks):
                    cw = cfg.cw[c]
                    lo, hi, rlists = chunk_tiles[c]
                    nS = hi - lo
                    msg = msgpool.tile([P, nS, H], bf16, tag="msg")
                    for s in range(nS):
                        nc.gpsimd.indirect_dma_start(
                            out=msg[:, s, :], out_offset=None,
                            in_=h_full[l][:],
                            in_offset=bass.IndirectOffsetOnAxis(
                                ap=idx_t[:, lo + s:lo + s + 1], axis=0))

                    mean = meanpool.tile([P, R, CHUNK], bf16, tag="mean")
                    for r in range(R):
                        psa = psA.tile([P, CHUNK], f32, tag="psA")
                        for (tb, tn, w) in rlists[r]:
                            ww = min(WIN, cw - w * WIN)
                            for t in range(tn):
                                s = tb + t
                                hot = hotpool.tile([P, WIN], bf16, tag="hot")
                                nc.vector.tensor_scalar(
                                    out=hot[:, :ww], in0=iota_bf[:, :ww],
                                    scalar1=loc_t[:, s:s + 1],
                                    scalar2=alp_t[:, s:s + 1],
                                    op0=OP.is_equal, op1=OP.mult)
                                nc.tensor.matmul(
                                    out=psa[:, w * WIN:w * WIN + ww],
                                    lhsT=msg[:, s - lo, :], rhs=hot[:, :ww],
                                    start=(t == 0), stop=(t == tn - 1))
                        if r % 2 == 0:
                            nc.vector.tensor_copy(mean[:, r, :cw], psa[:, :cw])
                        else:
                            nc.scalar.copy(mean[:, r, :cw], psa[:, :cw])

                    psb = psB.tile([P, CHUNK], f32, tag="psB")
                    nc.tensor.matmul(out=psb[:, :cw], lhsT=relw_t[:, root_i, :],
                                     rhs=hT[:, c * CHUNK:c * CHUNK + cw],
                                     start=True, stop=False)
                    for r in range(R):
                        nc.tensor.matmul(out=psb[:, :cw],
                                         lhsT=relw_t[:, l * R + r, :],
                                         rhs=mean[:, r, :cw],
                                         start=False, stop=(r == R - 1))

                    nc.vector.tensor_scalar(
                        out=outb[:, c * CHUNK:c * CHUNK + cw], in0=psb[:, :cw],
                        scalar1=1.0, scalar2=None, op0=OP.mult, op1=OP.add,
                        accum_out=sum_parts[:, c:c + 1])
                    nc.scalar.activation(sq_scr[:, :cw], psb[:, :cw], AF.Square,
                                         accum_out=sq_parts[:, c:c + 1])

                # ---------- BatchNorm + ReLU ----------
                st = workpool.tile([H, 2], f32, tag="stats")
                nc.vector.reduce_sum(st[:, 0:1], sum_parts[:],
                                     axis=mybir.AxisListType.X)
                nc.vector.reduce_sum(st[:, 1:2], sq_parts[:],
                                     axis=mybir.AxisListType.X)
                nc.sync.dma_start(stats_in[:], st[:])
                nc.gpsimd.collective_compute(
                    "AllReduce", OP.add, replica_groups=[cores],
                    ins=[stats_in[:]], outs=[stats_out[:]])
                stg = workpool.tile([H, 8], f32, tag="stg")
                nc.sync.dma_start(stg[:, 0:2], stats_out[:])
                nc.vector.tensor_scalar(out=stg[:, 2:3], in0=stg[:, 0:1],
                                        scalar1=1.0 / N, scalar2=None,
                                        op0=OP.mult)
                nc.vector.tensor_scalar(out=stg[:, 3:4], in0=stg[:, 1:2],
                                        scalar1=1.0 / N, scalar2=None,
                                        op0=OP.mult)
                nc.vector.tensor_tensor(out=stg[:, 4:5], in0=stg[:, 2:3],
                                        in1=stg[:, 2:3], op=OP.mult)
                nc.vector.tensor_tensor(out=stg[:, 4:5], in0=stg[:, 3:4],
                                        in1=stg[:, 4:5], op=OP.subtract)
                nc.vector.tensor_scalar(out=stg[:, 4:5], in0=stg[:, 4:5],
                                        scalar1=cfg.EPS, scalar2=None,
                                        op0=OP.add)
                nc.scalar.sqrt(stg[:, 5:6], stg[:, 4:5])
                nc.vector.reciprocal(stg[:, 6:7], stg[:, 5:6])
                nc.vector.tensor_tensor(out=stg[:, 6:7], in0=stg[:, 6:7],
                                        in1=bng_t[:, l:l + 1], op=OP.mult)
                nc.vector.tensor_tensor(out=stg[:, 7:8], in0=stg[:, 6:7],
                                        in1=stg[:, 2:3], op=OP.mult)
                nc.vector.tensor_tensor(out=stg[:, 7:8], in0=bnb_t[:, l:l + 1],
                                        in1=stg[:, 7:8], op=OP.subtract)
                if cfg.DEBUG:
                    nc.sync.dma_start(dbg_outb[l], outb[:])
                    nc.sync.dma_start(dbg_stg[l], stg[:])
                nc.scalar.activation(hT[:], outb[:], AF.Relu,
                                     bias=stg[:, 7:8], scale=stg[:, 6:7])

                if l + 1 < L:
                    emit_transpose_store(l + 1)

            # ---------- global mean pool + output MLP ----------
            psp = psB.tile([G, CHUNK], f32, tag="psB")
            for b in range(nblk):
                bw = min(P, NS - b * P)
                pst = psT.tile([P, P], bf16, tag="psT")
                nc.tensor.transpose(pst[:bw, :P], hT[:, b * P:b * P + bw],
                                    ident[:])
                rm = workpool.tile([P, P], bf16, tag="rm")
                nc.vector.tensor_copy(rm[:bw, :], pst[:bw, :P])
                ind = hotpool.tile([P, G], bf16, tag="ind")
                nc.vector.tensor_scalar(out=ind[:bw, :], in0=iota_bf[:bw, :G],
                                        scalar1=gids_t[:bw, b:b + 1],
                                        scalar2=None, op0=OP.is_equal)
                nc.tensor.matmul(out=psp[:, :H], lhsT=ind[:bw, :],
                                 rhs=rm[:bw, :], start=(b == 0),
                                 stop=(b == nblk - 1))
            poolt = workpool.tile([G, H], f32, tag="poolt")
            nc.vector.tensor_copy(poolt[:], psp[:, :H])
            nc.sync.dma_start(pool_in[:], poolt[:])
            nc.gpsimd.collective_compute(
                "AllReduce", OP.add, replica_groups=[cores],
                ins=[pool_in[:]], outs=[pool_out[:]])
            poolg = workpool.tile([G, H], f32, tag="poolg")
            nc.sync.dma_start(poolg[:], pool_out[:])
            if cfg.DEBUG:
                nc.gpsimd.dma_start(dbg_pool[:], pool_out[:])

            pstT = psT.tile([P, G], f32, tag="psTf")
            nc.tensor.transpose(pstT[:, :G], poolg[:], identf[:G, :G])
            poolT = workpool.tile([P, G], f32, tag="poolT")
            nc.vector.tensor_copy(poolT[:], pstT[:, :G])

            psl = psB.tile([C, CHUNK], f32, tag="psB")
            nc.tensor.matmul(out=psl[:, :G], lhsT=w_out_t[:], rhs=poolT[:],
                             start=True, stop=True)
            logit = workpool.tile([C, G], f32, tag="logit")
            nc.vector.tensor_tensor(out=logit[:], in0=psl[:, :G], in1=invg_t[:],
                                    op=OP.mult)
            logit2 = workpool.tile([C, G], f32, tag="logit2")
            nc.scalar.activation(logit2[:], logit[:], AF.Sigmoid,
                                 bias=b_out_t[:, 0:1], scale=1.0)
            nc.sync.dma_start(out_d[:], logit2[:])

    return nc


def _make_in_maps(cfg, plan, inputs):
    H, C, G, F, NS, R, L = cfg.H, cfg.C, cfg.G, cfg.F, cfg.NS, cfg.R, cfg.L
    x = np.asarray(inputs["x"], np.float32)
    batch = np.asarray(inputs["batch"])

    np_dt = BF16 if cfg.DT == "bf16" else np.float32
    relw = np.empty((L * R + L, P, H), np_dt)
    rel_w = np.asarray(inputs["rel_w"], np.float32)
    root_w = np.asarray(inputs["root_w"], np.float32)
    for l in range(L):
        for r in range(R):
            relw[l * R + r] = rel_w[l, r].astype(np_dt)
        relw[L * R + l] = root_w[l].astype(np_dt)

    bng = np.ascontiguousarray(np.asarray(inputs["bn_g"], np.float32).T)
    bnb = np.ascontiguousarray(np.asarray(inputs["bn_b"], np.float32).T)
    b_in = np.asarray(inputs["b_in"], np.float32).reshape(H, 1)
    b_out = np.asarray(inputs["b_out"], np.float32).reshape(C, 1)
    w_in = np.asarray(inputs["w_in"], np.float32)
    w_out = np.asarray(inputs["w_out"], np.float32)
    invg = np.ascontiguousarray(
        np.broadcast_to(plan["inv_gcnt"][None, :], (C, G)))
    relw_p = np.ascontiguousarray(relw.transpose(1, 0, 2))  # [P, 42, H]
    iota_np = np.broadcast_to(np.arange(cfg.WIN, dtype=np.float32)[None, :],
                              (P, cfg.WIN)).astype(BF16).copy()
    ident_np = np.eye(P, dtype=np.float32).astype(BF16)
    identf_np = np.eye(P, dtype=np.float32)

    nblk = math.ceil(NS / P)
    in_maps = []
    for c in range(cfg.NC):
        lo, hi = c * NS, (c + 1) * NS
        xT = np.ascontiguousarray(x[lo:hi].T)
        gids = np.full((P, nblk), -1.0, np.float32)
        bseg = batch[lo:hi].astype(np.float32)
        for b in range(nblk):
            bw = min(P, NS - b * P)
            gids[:bw, b] = bseg[b * P:b * P + bw]
        in_maps.append(dict(
            xT=xT, idxA=np.ascontiguousarray(plan["idxA"][c]),
            iotain=iota_np, identb=ident_np, identf=identf_np,
            locA=np.ascontiguousarray(plan["locA"][c]),
            alphaA=np.ascontiguousarray(plan["alphaA"][c]),
            w_in=w_in, b_in=b_in, relw=relw_p, bng=bng, bnb=bnb,
            w_out=w_out, b_out=b_out, gids=gids, invg=invg,
        ))
    return in_maps


def _plan_key(cfg, plan):
    def _freeze(x):
        if isinstance(x, (list, tuple)):
            return tuple(_freeze(v) for v in x)
        return x
    return (cfg.N, cfg.E, cfg.F, cfg.H, cfg.R, cfg.G, cfg.C, cfg.L, cfg.NC,
            cfg.CHUNK, cfg.WIN, cfg.DT, cfg.DEBUG, plan["S_total"],
            _freeze(plan["chunk_tiles"]))


class _Runner:
    """Compile-once PJRT runner (mirrors run_bass_kernel_spmd's axon path,
    but keeps the jitted callable so repeat invocations skip retrace,
    XLA recompile, and executable reload)."""

    def __init__(self, cfg, plan):
        import jax
        from jax.sharding import Mesh, PartitionSpec, NamedSharding
        try:
            from jax.experimental.shard_map import shard_map
        except ImportError:
            from jax import shard_map
        from concourse.bass2jax import (
            _bass_exec_p, partition_id_tensor, install_neuronx_cc_hook)

        self.cfg = cfg
        nc = _build_nc(cfg, plan)
        if not nc.is_finalized():
            nc.finalize()
        self.nc = nc
        install_neuronx_cc_hook()
        assert nc.dbg_addr is None

        pname = nc.partition_id_tensor.name if nc.partition_id_tensor else None
        in_names, out_names, out_avals, self.zero_shapes = [], [], [], []
        for alloc in nc.m.functions[0].allocations:
            if not isinstance(alloc, mybir.MemoryLocationSet):
                continue
            name = alloc.memorylocations[0].name
            if alloc.kind == "ExternalInput":
                if name != pname:
                    in_names.append(name)
            elif alloc.kind == "ExternalOutput":
                out_names.append(name)
                shape = tuple(alloc.tensor_shape)
                dtype = mybir.dt.np(alloc.dtype)
                out_avals.append(jax.core.ShapedArray(shape, dtype))
                self.zero_shapes.append((shape, dtype))
        self.in_names, self.out_names = in_names, out_names
        self.out_avals = out_avals
        n_params, n_outs = len(in_names), len(out_avals)
        all_names = in_names + out_names + ([pname] if pname else [])
        donate = tuple(range(n_params, n_params + n_outs))

        def _body(*args):
            operands = list(args)
            if pname is not None:
                operands.append(partition_id_tensor())
            return tuple(_bass_exec_p.bind(
                *operands, out_avals=tuple(out_avals),
                in_names=tuple(all_names), out_names=tuple(out_names),
                lowering_input_output_aliases=(),
                sim_require_finite=True, sim_require_nnan=True, nc=nc))

        devices = jax.devices()[:cfg.NC]
        assert len(devices) == cfg.NC
        self.mesh = Mesh(np.asarray(devices), ("core",))
        self.shard = NamedSharding(self.mesh, PartitionSpec("core"))
        self.jit = jax.jit(
            shard_map(_body, mesh=self.mesh,
                      in_specs=(PartitionSpec("core"),) * (n_params + n_outs),
                      out_specs=(PartitionSpec("core"),) * n_outs,
                      check_rep=False),
            donate_argnums=donate, keep_unused=True)

    def concat_inputs(self, in_maps):
        return [np.concatenate([np.asarray(m[n]) for m in in_maps], axis=0)
                for n in self.in_names]

    def put_inputs(self, concat):
        import jax
        dev = [jax.device_put(a, self.shard) for a in concat]
        jax.block_until_ready(dev)
        return dev

    def zeros(self, device=False):
        import jax
        zs = [np.zeros((self.cfg.NC * s[0], *s[1:]), dt)
              for (s, dt) in self.zero_shapes]
        if device:
            zs = [jax.device_put(z, self.shard) for z in zs]
            jax.block_until_ready(zs)
        return zs

    def __call__(self, ins, zeros):
        return self.jit(*ins, *zeros)

    def out_core0(self, outs):
        i = self.out_names.index("out")
        shape = self.out_avals[i].shape
        return np.asarray(outs[i]).reshape(self.cfg.NC, *shape)[0]


_RUNNER_CACHE = {}


def _get_runner(cfg, plan):
    key = _plan_key(cfg, plan)
    r = _RUNNER_CACHE.get(key)
    if r is None:
        r = _Runner(cfg, plan)
        _RUNNER_CACHE[key] = r
    return r


def _run(cfg, inputs, **kw):
    plan = _plan(cfg, np.asarray(inputs["edge_index"]),
                 np.asarray(inputs["edge_type"]), np.asarray(inputs["batch"]))
    runner = _get_runner(cfg, plan)
    in_maps = _make_in_maps(cfg, plan, inputs)
    concat = runner.concat_inputs(in_maps)
    outs = runner(concat, runner.zeros())
    out = runner.out_core0(outs)
    return np.ascontiguousarray(np.asarray(out).T.astype(np.float32)), runner


def kernel(**inputs):
    cfg = Cfg()
    out, _ = _run(cfg, inputs)
    return out



# revision 13
# speedup vs baseline: 273.2433x; 1.1283x over previous
"""Trainium2 Bass kernel for 2-layer RGCN (nn_PygModel_52003464020165).

Self-contained: accepts FULL inputs, shards across 8 NeuronCores internally,
returns FULL [64, 10] output.

Architecture (per core, dst-sharded graph):
  - full h replicated each layer via AllGather (bf16, [N, H] row-major in DRAM)
  - per dst-chunk (512 dense dst cols): batched indirect-DMA gather of
    h[src] rows -> msg tiles [128 edges, H] (edges on partitions)
  - per relation r: alpha-hot matrices [128 edges, 128 win] built by one DVE
    tensor_scalar (is_equal vs iota, scaled by 1/cnt); PE matmuls
    msg^T @ alphahot accumulate mean bins into PSUM [H, chunk]
  - transform: root matmul + 20 relation matmuls (W_r stationary, bf16)
    accumulate out^T [H, chunk] in PSUM; evacuation fuses BN partial stats
  - BatchNorm stats via AllReduce; affine+ReLU as one ACT op over [H, NS]
  - PE transposes h^T -> row-major shard -> DRAM -> AllGather
  - global mean pool via indicator matmuls + AllReduce; final linear+sigmoid
"""

import math
import sys

sys.path.insert(0, "/opt/trn_rl_repo")

import ml_dtypes
import numpy as np

import concourse.bacc as bacc
import concourse.bass as bass
import concourse.tile as tile
from concourse import mybir
from concourse import library_config
from concourse.bass_utils import run_bass_kernel_spmd

BF16 = ml_dtypes.bfloat16
P = 128


class Cfg:
    def __init__(self, N=100000, E=1600000, F=64, H=128, R=20, G=64, C=10, L=2,
                 NC=8, CHUNK=1024, WIN=128, EPS=1e-5, DT="bf16", DEBUG=False,
                 RSZ=25000):
        assert H == P
        self.N, self.E, self.F, self.H, self.R, self.G, self.C, self.L = (
            N, E, F, H, R, G, C, L)
        self.NC, self.CHUNK, self.WIN, self.EPS = NC, CHUNK, WIN, EPS
        self.DT = DT
        self.DEBUG = DEBUG
        self.RSZ = RSZ
        self.NREG = math.ceil(N / RSZ)
        assert N % NC == 0
        self.NS = N // NC
        self.nchunks = math.ceil(self.NS / CHUNK)
        self.cw = [min(CHUNK, self.NS - c * CHUNK) for c in range(self.nchunks)]
        self.nwin = [math.ceil(w / WIN) for w in self.cw]
        self.SENT = N  # sentinel gather index (> N-1 bound -> skipped)


def _plan(cfg, edge_index, edge_type, batch):
    """Host-side planner. Returns shared structure + per-core data arrays."""
    N, R, NC, NS, CHUNK, WIN = cfg.N, cfg.R, cfg.NC, cfg.NS, cfg.CHUNK, cfg.WIN
    src = edge_index[0].astype(np.int64)
    dst = edge_index[1].astype(np.int64)
    et = edge_type.astype(np.int64)

    comb = dst * R + et
    cnt = np.bincount(comb, minlength=N * R).astype(np.float64)
    alpha_e = (1.0 / np.maximum(cnt[comb], 1.0)).astype(np.float32)

    core = dst // NS
    dloc = dst % NS
    chunk = dloc // CHUNK
    inchunk = dloc % CHUNK
    win = inchunk // WIN
    loc = (inchunk % WIN).astype(np.float32)

    maxwin = max(cfg.nwin)
    gid = (chunk * R + et) * maxwin + win
    ngroups = cfg.nchunks * R * maxwin

    counts = np.zeros((NC, ngroups), np.int64)
    np.add.at(counts, (core, gid), 1)
    Tg = np.maximum(1, -(-counts.max(axis=0) // P))  # ceil div, min 1

    # tile order: chunk-major, then r, then win
    group_order = []
    for c in range(cfg.nchunks):
        for r in range(R):
            for w in range(cfg.nwin[c]):
                group_order.append((c * R + r) * maxwin + w)
    group_order = np.array(group_order, np.int64)
    tiles_of_group = Tg[group_order]
    tile_base = np.zeros(len(group_order), np.int64)
    np.cumsum(tiles_of_group[:-1], out=tile_base[1:])
    S_total = int(tiles_of_group.sum())

    gpos = np.full(ngroups, -1, np.int64)
    gpos[group_order] = np.arange(len(group_order))

    locA = np.full((NC, P, S_total), -1.0, np.float32)
    alphaA = np.zeros((NC, P, S_total), np.float32)
    srcA = np.zeros((NC, P, S_total), np.int64)  # src per slot (sentinel: -1)
    srcA[:] = -1

    order = np.lexsort((gid, core))
    s_core, s_gid = core[order], gid[order]
    s_src, s_loc, s_alpha = src[order], loc[order], alpha_e[order]
    key = s_core * ngroups + s_gid
    first = np.r_[True, key[1:] != key[:-1]]
    grp_start = np.flatnonzero(first)
    seglen = np.diff(np.r_[grp_start, len(key)])
    rank = np.arange(len(key)) - np.repeat(grp_start, seglen)

    slot = tile_base[gpos[s_gid]] * P + rank
    srcA[s_core, slot % P, slot // P] = s_src
    locA[s_core, slot % P, slot // P] = s_loc
    alphaA[s_core, slot % P, slot // P] = s_alpha

    # emission structure: per chunk -> (slot_lo, slot_hi,
    #   per-r list of per-win (tile_base, ntiles))
    chunk_tiles = []
    for c in range(cfg.nchunks):
        lo = None
        hi = 0
        rlists = []
        for r in range(R):
            wl = []
            for w in range(cfg.nwin[c]):
                pos = gpos[(c * R + r) * maxwin + w]
                tb, tn = int(tile_base[pos]), int(tiles_of_group[pos])
                if lo is None:
                    lo = tb
                hi = tb + tn
                wl.append((tb, tn, w))
            rlists.append(wl)
        chunk_tiles.append((lo, hi, rlists))

    idxA = np.where(srcA >= 0, srcA, 0).astype(np.int32)

    gcnt = np.bincount(batch.astype(np.int64), minlength=cfg.G).astype(np.float32)
    inv_gcnt = 1.0 / np.maximum(gcnt, 1.0)

    return dict(S_total=S_total, chunk_tiles=chunk_tiles, locA=locA,
                alphaA=alphaA, inv_gcnt=inv_gcnt, idxA=idxA)


def _build_nc(cfg, plan):
    """Emit the SPMD Bass program (one program, NC cores)."""
    N, F, H, R, G, C, L = cfg.N, cfg.F, cfg.H, cfg.R, cfg.G, cfg.C, cfg.L
    NS, CHUNK, WIN = cfg.NS, cfg.CHUNK, cfg.WIN
    S_total = plan["S_total"]
    chunk_tiles = plan["chunk_tiles"]
    nblk = math.ceil(NS / P)

    nc = bacc.Bacc(None)
    f32, i32, i16 = mybir.dt.float32, mybir.dt.int32, mybir.dt.int16
    bf16 = mybir.dt.bfloat16 if cfg.DT == "bf16" else mybir.dt.float32
    AF = mybir.ActivationFunctionType
    OP = mybir.AluOpType

    xT_d = nc.dram_tensor("xT", [F, NS], f32, kind="ExternalInput")
    idx_d = nc.dram_tensor("idxA", [P, S_total], i32, kind="ExternalInput")
    iota_d = nc.dram_tensor("iotain", [P, WIN], bf16, kind="ExternalInput")
    identb_d = nc.dram_tensor("identb", [P, P], bf16, kind="ExternalInput")
    identf_d = nc.dram_tensor("identf", [P, P], f32, kind="ExternalInput")
    loc_d = nc.dram_tensor("locA", [P, S_total], f32, kind="ExternalInput")
    alp_d = nc.dram_tensor("alphaA", [P, S_total], f32, kind="ExternalInput")
    w_in_d = nc.dram_tensor("w_in", [F, H], f32, kind="ExternalInput")
    b_in_d = nc.dram_tensor("b_in", [H, 1], f32, kind="ExternalInput")
    relw_d = nc.dram_tensor("relw", [P, L * R + L, H], bf16, kind="ExternalInput")
    bng_d = nc.dram_tensor("bng", [H, L], f32, kind="ExternalInput")
    bnb_d = nc.dram_tensor("bnb", [H, L], f32, kind="ExternalInput")
    w_out_d = nc.dram_tensor("w_out", [H, C], f32, kind="ExternalInput")
    b_out_d = nc.dram_tensor("b_out", [C, 1], f32, kind="ExternalInput")
    gids_d = nc.dram_tensor("gids", [P, nblk], f32, kind="ExternalInput")
    invg_d = nc.dram_tensor("invg", [C, G], f32, kind="ExternalInput")
    out_d = nc.dram_tensor("out", [C, G], f32, kind="ExternalOutput")

    h_shard = [nc.dram_tensor(f"h_shard{l}", [NS, H], bf16) for l in range(L)]
    h_full = [nc.dram_tensor(f"h_full{l}", [N, H], bf16, addr_space="Shared")
              for l in range(L)]
    stats_in = nc.dram_tensor("stats_in", [H, 2], f32)
    stats_out = nc.dram_tensor("stats_out", [H, 2], f32, addr_space="Shared")
    pool_in = nc.dram_tensor("pool_in", [G, H], f32)
    pool_out = nc.dram_tensor("pool_out", [G, H], f32, addr_space="Shared")
    if cfg.DEBUG:
        dbg_h = [nc.dram_tensor(f"dbg_h{l}", [N, H], bf16,
                                kind="ExternalOutput") for l in range(L)]
        dbg_outb = nc.dram_tensor("dbg_outb", [L, H, NS], bf16,
                                  kind="ExternalOutput")
        dbg_stg = nc.dram_tensor("dbg_stg", [L, H, 8], f32,
                                 kind="ExternalOutput")
        dbg_pool = nc.dram_tensor("dbg_pool", [G, H], f32,
                                  kind="ExternalOutput")

    cores = list(range(cfg.NC))

    with tile.TileContext(nc) as tc:
        with (
            tc.tile_pool(name="const", bufs=1) as cpool,
            tc.tile_pool(name="big", bufs=1) as bigpool,
            tc.tile_pool(name="msg", bufs=32) as msgpool,
            tc.tile_pool(name="hot", bufs=16) as hotpool,
            tc.tile_pool(name="mean", bufs=1) as meanpool,
            tc.tile_pool(name="work", bufs=3) as workpool,
            tc.tile_pool(name="psA", bufs=2, space="PSUM") as psA,
            tc.tile_pool(name="psT", bufs=1, space="PSUM") as psT,
            tc.tile_pool(name="psB", bufs=1, space="PSUM") as psB,
        ):
            # ---------- constants ----------
            iota_bf = cpool.tile([P, WIN], bf16, tag="iota_bf")
            nc.sync.dma_start(iota_bf[:], iota_d[:])
            ident = cpool.tile([P, P], bf16, tag="ident")
            nc.sync.dma_start(ident[:], identb_d[:])
            identf = cpool.tile([P, P], f32, tag="identf")
            nc.sync.dma_start(identf[:], identf_d[:])

            idx_t = cpool.tile([P, S_total], i32, tag="idx")
            nc.sync.dma_start(idx_t[:], idx_d[:])
            loc_t = cpool.tile([P, S_total], f32, tag="loc")
            nc.sync.dma_start(loc_t[:], loc_d[:])
            alp_t = cpool.tile([P, S_total], f32, tag="alp")
            nc.sync.dma_start(alp_t[:], alp_d[:])

            relw_t = cpool.tile([P, L * R + L, H], bf16, tag="relw")
            nc.sync.dma_start(relw_t[:], relw_d[:])
            w_in_t = cpool.tile([F, H], f32, tag="w_in")
            nc.sync.dma_start(w_in_t[:], w_in_d[:])
            b_in_t = cpool.tile([H, 1], f32, tag="b_in")
            nc.sync.dma_start(b_in_t[:], b_in_d[:])
            bng_t = cpool.tile([H, L], f32, tag="bng")
            nc.sync.dma_start(bng_t[:], bng_d[:])
            bnb_t = cpool.tile([H, L], f32, tag="bnb")
            nc.sync.dma_start(bnb_t[:], bnb_d[:])
            gids_t = cpool.tile([P, nblk], f32, tag="gids")
            nc.sync.dma_start(gids_t[:], gids_d[:])
            w_out_t = cpool.tile([H, C], f32, tag="w_out")
            nc.sync.dma_start(w_out_t[:], w_out_d[:])
            b_out_t = cpool.tile([C, 1], f32, tag="b_out")
            nc.sync.dma_start(b_out_t[:], b_out_d[:])
            invg_t = cpool.tile([C, G], f32, tag="invg")
            nc.sync.dma_start(invg_t[:], invg_d[:])

            # fences: pull const-load DMA completions into engine program
            # order one DMA at a time, so compute ops (tiny ISA wait
            # budgets) emit no DMA waits of their own
            fence = cpool.tile([1, 1], f32, tag="fence")
            for _ft in (gids_t, bng_t, bnb_t, invg_t, w_in_t, w_out_t):
                nc.vector.tensor_copy(fence[:], _ft[0:1, 0:1].bitcast(f32))
            for _ft in (loc_t, alp_t):
                nc.vector.tensor_copy(fence[:], _ft[0:1, 0:1].bitcast(f32))
            fenceA = cpool.tile([1, 1], f32, tag="fenceA")
            for _ft in (b_in_t, b_out_t):
                nc.scalar.copy(fenceA[:], _ft[0:1, 0:1])
            _rw = 2 if cfg.DT == "bf16" else 1
            nc.scalar.copy(fenceA[:], relw_t[0:1, 0, 0:_rw].bitcast(f32))

            hT = bigpool.tile([P, NS], bf16, tag="hT")
            outb = bigpool.tile([P, NS], bf16, tag="outb")
            sum_parts = bigpool.tile([P, cfg.nchunks], f32, tag="sumP")
            sq_parts = bigpool.tile([P, cfg.nchunks], f32, tag="sqP")
            sq_scr = bigpool.tile([P, CHUNK], bf16, tag="sqscr")

            # ---------- input MLP ----------
            for c in range(cfg.nchunks):
                cw = cfg.cw[c]
                xc = workpool.tile([F, CHUNK], f32, tag="xc")
                nc.sync.dma_start(xc[:, :cw], xT_d[:, c * CHUNK:c * CHUNK + cw])
                ps = psB.tile([P, CHUNK], f32, tag="psB")
                for h0 in range(0, cw, 512):
                    hw_ = min(512, cw - h0)
                    nc.tensor.matmul(out=ps[:, h0:h0 + hw_], lhsT=w_in_t[:],
                                     rhs=xc[:, h0:h0 + hw_],
                                     start=True, stop=True)
                nc.scalar.activation(hT[:, c * CHUNK:c * CHUNK + cw], ps[:, :cw],
                                     AF.Relu, bias=b_in_t[:, 0:1], scale=1.0)

            def emit_transpose_store(l):
                for b in range(nblk):
                    bw = min(P, NS - b * P)
                    pst = psT.tile([P, P], bf16, tag="psT")
                    nc.tensor.transpose(pst[:bw, :P], hT[:, b * P:b * P + bw],
                                        ident[:])
                    rm = workpool.tile([P, P], bf16, tag="rm")
                    nc.vector.tensor_copy(rm[:bw, :], pst[:bw, :P])
                    nc.sync.dma_start(h_shard[l][b * P:b * P + bw, :], rm[:bw, :])
                nc.gpsimd.collective_compute(
                    "AllGather", OP.bypass, replica_groups=[cores],
                    ins=[h_shard[l][:]], outs=[h_full[l][:]])
                if cfg.DEBUG:
                    nc.gpsimd.dma_start(dbg_h[l][:], h_full[l][:])

            emit_transpose_store(0)

            # ---------- RGCN layers ----------
            for l in range(L):
                root_i = L * R + l
                for c in range(cfg.nchunks):
                    cw = cfg.cw[c]
                    lo, hi, rlists = chunk_tiles[c]
                    mean = meanpool.tile([P, R, CHUNK], bf16, tag="mean")
                    for r in range(R):
                        psa = psA.tile([P, CHUNK], f32, tag="psA")
                        for (tb, tn, w) in rlists[r]:
                            ww = min(WIN, cw - w * WIN)
                            for t in range(tn):
                                s = tb + t
                                m = msgpool.tile([P, H], bf16, tag="msg")
                                nc.gpsimd.indirect_dma_start(
                                    out=m[:], out_offset=None,
                                    in_=h_full[l][:],
                                    in_offset=bass.IndirectOffsetOnAxis(
                                        ap=idx_t[:, s:s + 1], axis=0))
                                hot = hotpool.tile([P, WIN], bf16, tag="hot")
                                nc.vector.tensor_scalar(
                                    out=hot[:, :ww], in0=iota_bf[:, :ww],
                                    scalar1=loc_t[:, s:s + 1],
                                    scalar2=alp_t[:, s:s + 1],
                                    op0=OP.is_equal, op1=OP.mult)
                                nc.tensor.matmul(
                                    out=psa[:, w * WIN:w * WIN + ww],
                                    lhsT=m[:], rhs=hot[:, :ww],
                                    start=(t == 0), stop=(t == tn - 1))
                        if r % 2 == 0:
                            nc.vector.tensor_copy(mean[:, r, :cw], psa[:, :cw])
                        else:
                            nc.scalar.copy(mean[:, r, :cw], psa[:, :cw])

                    psb = psB.tile([P, CHUNK], f32, tag="psB")
                    # split transform at the 512-col PSUM bank boundary
                    for h0 in range(0, cw, 512):
                        hw_ = min(512, cw - h0)
                        nc.tensor.matmul(
                            out=psb[:, h0:h0 + hw_], lhsT=relw_t[:, root_i, :],
                            rhs=hT[:, c * CHUNK + h0:c * CHUNK + h0 + hw_],
                            start=True, stop=False)
                        for r in range(R):
                            nc.tensor.matmul(out=psb[:, h0:h0 + hw_],
                                             lhsT=relw_t[:, l * R + r, :],
                                             rhs=mean[:, r, h0:h0 + hw_],
                                             start=False, stop=(r == R - 1))

                    nc.vector.tensor_scalar(
                        out=outb[:, c * CHUNK:c * CHUNK + cw], in0=psb[:, :cw],
                        scalar1=1.0, scalar2=None, op0=OP.mult, op1=OP.add,
                        accum_out=sum_parts[:, c:c + 1])
                    nc.scalar.activation(sq_scr[:, :cw], psb[:, :cw], AF.Square,
                                         accum_out=sq_parts[:, c:c + 1])

                # ---------- BatchNorm + ReLU ----------
                st = workpool.tile([H, 2], f32, tag="stats")
                nc.vector.reduce_sum(st[:, 0:1], sum_parts[:],
                                     axis=mybir.AxisListType.X)
                nc.vector.reduce_sum(st[:, 1:2], sq_parts[:],
                                     axis=mybir.AxisListType.X)
                nc.sync.dma_start(stats_in[:], st[:])
                nc.gpsimd.collective_compute(
                    "AllReduce", OP.add, replica_groups=[cores],
                    ins=[stats_in[:]], outs=[stats_out[:]])
                stg = workpool.tile([H, 8], f32, tag="stg")
                nc.sync.dma_start(stg[:, 0:2], stats_out[:])
                nc.vector.tensor_scalar(out=stg[:, 2:3], in0=stg[:, 0:1],
                                        scalar1=1.0 / N, scalar2=None,
                                        op0=OP.mult)
                nc.vector.tensor_scalar(out=stg[:, 3:4], in0=stg[:, 1:2],
                                        scalar1=1.0 / N, scalar2=None,
                                        op0=OP.mult)
                nc.vector.tensor_tensor(out=stg[:, 4:5], in0=stg[:, 2:3],
                                        in1=stg[:, 2:3], op=OP.mult)
                nc.vector.tensor_tensor(out=stg[:, 4:5], in0=stg[:, 3:4],
                                        in1=stg[:, 4:5], op=OP.subtract)
                nc.vector.tensor_scalar(out=stg[:, 4:5], in0=stg[:, 4:5],
                                        scalar1=cfg.EPS, scalar2=None,
                                        op0=OP.add)
                nc.scalar.sqrt(stg[:, 5:6], stg[:, 4:5])
                nc.vector.reciprocal(stg[:, 6:7], stg[:, 5:6])
                nc.vector.tensor_tensor(out=stg[:, 6:7], in0=stg[:, 6:7],
                                        in1=bng_t[:, l:l + 1], op=OP.mult)
                nc.vector.tensor_tensor(out=stg[:, 7:8], in0=stg[:, 6:7],
                                        in1=stg[:, 2:3], op=OP.mult)
                nc.vector.tensor_tensor(out=stg[:, 7:8], in0=bnb_t[:, l:l + 1],
                                        in1=stg[:, 7:8], op=OP.subtract)
                if cfg.DEBUG:
                    nc.sync.dma_start(dbg_outb[l], outb[:])
                    nc.sync.dma_start(dbg_stg[l], stg[:])
                nc.scalar.activation(hT[:], outb[:], AF.Relu,
                                     bias=stg[:, 7:8], scale=stg[:, 6:7])

                if l + 1 < L:
                    emit_transpose_store(l + 1)

            # ---------- global mean pool + output MLP ----------
            psp = psB.tile([G, CHUNK], f32, tag="psB")
            for b in range(nblk):
                bw = min(P, NS - b * P)
                pst = psT.tile([P, P], bf16, tag="psT")
                nc.tensor.transpose(pst[:bw, :P], hT[:, b * P:b * P + bw],
                                    ident[:])
                rm = workpool.tile([P, P], bf16, tag="rm")
                nc.vector.tensor_copy(rm[:bw, :], pst[:bw, :P])
                ind = hotpool.tile([P, G], bf16, tag="ind")
                nc.vector.tensor_scalar(out=ind[:bw, :], in0=iota_bf[:bw, :G],
                                        scalar1=gids_t[:bw, b:b + 1],
                                        scalar2=None, op0=OP.is_equal)
                nc.tensor.matmul(out=psp[:, :H], lhsT=ind[:bw, :],
                                 rhs=rm[:bw, :], start=(b == 0),
                                 stop=(b == nblk - 1))
            poolt = workpool.tile([G, H], f32, tag="poolt")
            nc.vector.tensor_copy(poolt[:], psp[:, :H])
            nc.sync.dma_start(pool_in[:], poolt[:])
            nc.gpsimd.collective_compute(
                "AllReduce", OP.add, replica_groups=[cores],
                ins=[pool_in[:]], outs=[pool_out[:]])
            poolg = workpool.tile([G, H], f32, tag="poolg")
            nc.sync.dma_start(poolg[:], pool_out[:])
            if cfg.DEBUG:
                nc.gpsimd.dma_start(dbg_pool[:], pool_out[:])

            pstT = psT.tile([P, G], f32, tag="psTf")
            nc.tensor.transpose(pstT[:, :G], poolg[:], identf[:G, :G])
            poolT = workpool.tile([P, G], f32, tag="poolT")
            nc.vector.tensor_copy(poolT[:], pstT[:, :G])

            psl = psB.tile([C, CHUNK], f32, tag="psB")
            nc.tensor.matmul(out=psl[:, :G], lhsT=w_out_t[:], rhs=poolT[:],
                             start=True, stop=True)
            logit = workpool.tile([C, G], f32, tag="logit")
            nc.vector.tensor_tensor(out=logit[:], in0=psl[:, :G], in1=invg_t[:],
                                    op=OP.mult)
            logit2 = workpool.tile([C, G], f32, tag="logit2")
            nc.scalar.activation(logit2[:], logit[:], AF.Sigmoid,
                                 bias=b_out_t[:, 0:1], scale=1.0)
            nc.sync.dma_start(out_d[:], logit2[:])

    return nc


def _make_in_maps(cfg, plan, inputs):
    H, C, G, F, NS, R, L = cfg.H, cfg.C, cfg.G, cfg.F, cfg.NS, cfg.R, cfg.L
    x = np.asarray(inputs["x"], np.float32)
    batch = np.asarray(inputs["batch"])

    np_dt = BF16 if cfg.DT == "bf16" else np.float32
    relw = np.empty((L * R + L, P, H), np_dt)
    rel_w = np.asarray(inputs["rel_w"], np.float32)
    root_w = np.asarray(inputs["root_w"], np.float32)
    for l in range(L):
        for r in range(R):
            relw[l * R + r] = rel_w[l, r].astype(np_dt)
        relw[L * R + l] = root_w[l].astype(np_dt)

    bng = np.ascontiguousarray(np.asarray(inputs["bn_g"], np.float32).T)
    bnb = np.ascontiguousarray(np.asarray(inputs["bn_b"], np.float32).T)
    b_in = np.asarray(inputs["b_in"], np.float32).reshape(H, 1)
    b_out = np.asarray(inputs["b_out"], np.float32).reshape(C, 1)
    w_in = np.asarray(inputs["w_in"], np.float32)
    w_out = np.asarray(inputs["w_out"], np.float32)
    invg = np.ascontiguousarray(
        np.broadcast_to(plan["inv_gcnt"][None, :], (C, G)))
    relw_p = np.ascontiguousarray(relw.transpose(1, 0, 2))  # [P, 42, H]
    iota_np = np.broadcast_to(np.arange(cfg.WIN, dtype=np.float32)[None, :],
                              (P, cfg.WIN)).astype(BF16).copy()
    ident_np = np.eye(P, dtype=np.float32).astype(BF16)
    identf_np = np.eye(P, dtype=np.float32)

    nblk = math.ceil(NS / P)
    in_maps = []
    for c in range(cfg.NC):
        lo, hi = c * NS, (c + 1) * NS
        xT = np.ascontiguousarray(x[lo:hi].T)
        gids = np.full((P, nblk), -1.0, np.float32)
        bseg = batch[lo:hi].astype(np.float32)
        for b in range(nblk):
            bw = min(P, NS - b * P)
            gids[:bw, b] = bseg[b * P:b * P + bw]
        in_maps.append(dict(
            xT=xT, idxA=np.ascontiguousarray(plan["idxA"][c]),
            iotain=iota_np, identb=ident_np, identf=identf_np,
            locA=np.ascontiguousarray(plan["locA"][c]),
            alphaA=np.ascontiguousarray(plan["alphaA"][c]),
            w_in=w_in, b_in=b_in, relw=relw_p, bng=bng, bnb=bnb,
            w_out=w_out, b_out=b_out, gids=gids, invg=invg,
        ))
    return in_maps


def _plan_key(cfg, plan):
    def _freeze(x):
        if isinstance(x, (list, tuple)):
            return tuple(_freeze(v) for v in x)
        return x
    return (cfg.N, cfg.E, cfg.F, cfg.H, cfg.R, cfg.G, cfg.C, cfg.L, cfg.NC,
            cfg.CHUNK, cfg.WIN, cfg.DT, cfg.DEBUG, plan["S_total"],
            _freeze(plan["chunk_tiles"]))


class _Runner:
    """Compile-once PJRT runner (mirrors run_bass_kernel_spmd's axon path,
    but keeps the jitted callable so repeat invocations skip retrace,
    XLA recompile, and executable reload)."""

    def __init__(self, cfg, plan):
        import jax
        from jax.sharding import Mesh, PartitionSpec, NamedSharding
        try:
            from jax.experimental.shard_map import shard_map
        except ImportError:
            from jax import shard_map
        from concourse.bass2jax import (
            _bass_exec_p, partition_id_tensor, install_neuronx_cc_hook)

        self.cfg = cfg
        nc = _build_nc(cfg, plan)
        if not nc.is_finalized():
            nc.finalize()
        self.nc = nc
        install_neuronx_cc_hook()
        assert nc.dbg_addr is None

        pname = nc.partition_id_tensor.name if nc.partition_id_tensor else None
        in_names, out_names, out_avals, self.zero_shapes = [], [], [], []
        for alloc in nc.m.functions[0].allocations:
            if not isinstance(alloc, mybir.MemoryLocationSet):
                continue
            name = alloc.memorylocations[0].name
            if alloc.kind == "ExternalInput":
                if name != pname:
                    in_names.append(name)
            elif alloc.kind == "ExternalOutput":
                out_names.append(name)
                shape = tuple(alloc.tensor_shape)
                dtype = mybir.dt.np(alloc.dtype)
                out_avals.append(jax.core.ShapedArray(shape, dtype))
                self.zero_shapes.append((shape, dtype))
        self.in_names, self.out_names = in_names, out_names
        self.out_avals = out_avals
        n_params, n_outs = len(in_names), len(out_avals)
        all_names = in_names + out_names + ([pname] if pname else [])
        donate = tuple(range(n_params, n_params + n_outs))

        def _body(*args):
            operands = list(args)
            if pname is not None:
                operands.append(partition_id_tensor())
            return tuple(_bass_exec_p.bind(
                *operands, out_avals=tuple(out_avals),
                in_names=tuple(all_names), out_names=tuple(out_names),
                lowering_input_output_aliases=(),
                sim_require_finite=True, sim_require_nnan=True, nc=nc))

        devices = jax.devices()[:cfg.NC]
        assert len(devices) == cfg.NC
        self.mesh = Mesh(np.asarray(devices), ("core",))
        self.shard = NamedSharding(self.mesh, PartitionSpec("core"))
        self.jit = jax.jit(
            shard_map(_body, mesh=self.mesh,
                      in_specs=(PartitionSpec("core"),) * (n_params + n_outs),
                      out_specs=(PartitionSpec("core"),) * n_outs,
                      check_rep=False),
            donate_argnums=donate, keep_unused=True)

    def concat_inputs(self, in_maps):
        return [np.concatenate([np.asarray(m[n]) for m in in_maps], axis=0)
                for n in self.in_names]

    def put_inputs(self, concat):
        import jax
        dev = [jax.device_put(a, self.shard) for a in concat]
        jax.block_until_ready(dev)
        return dev

    def zeros(self, device=False):
        import jax
        zs = [np.zeros((self.cfg.NC * s[0], *s[1:]), dt)
              for (s, dt) in self.zero_shapes]
        if device:
            zs = [jax.device_put(z, self.shard) for z in zs]
            jax.block_until_ready(zs)
        return zs

    def __call__(self, ins, zeros):
        return self.jit(*ins, *zeros)

    def out_core0(self, outs):
        i = self.out_names.index("out")
        shape = self.out_avals[i].shape
        return np.asarray(outs[i]).reshape(self.cfg.NC, *shape)[0]


_RUNNER_CACHE = {}


def _get_runner(cfg, plan):
    key = _plan_key(cfg, plan)
    r = _RUNNER_CACHE.get(key)
    if r is None:
        r = _Runner(cfg, plan)
        _RUNNER_CACHE[key] = r
    return r


def _run(cfg, inputs, **kw):
    plan = _plan(cfg, np.asarray(inputs["edge_index"]),
                 np.asarray(inputs["edge_type"]), np.asarray(inputs["batch"]))
    runner = _get_runner(cfg, plan)
    in_maps = _make_in_maps(cfg, plan, inputs)
    concat = runner.concat_inputs(in_maps)
    outs = runner(concat, runner.zeros())
    out = runner.out_core0(outs)
    return np.ascontiguousarray(np.asarray(out).T.astype(np.float32)), runner


def kernel(**inputs):
    cfg = Cfg()
    out, _ = _run(cfg, inputs)
    return out



# revision 15
# speedup vs baseline: 296.2146x; 1.0841x over previous
"""Trainium2 Bass kernel for 2-layer RGCN (nn_PygModel_52003464020165).

Self-contained: accepts FULL inputs, shards across 8 NeuronCores internally,
returns FULL [64, 10] output.

Architecture (per core, dst-sharded graph):
  - full h replicated each layer via AllGather (bf16, [N, H] row-major in DRAM)
  - per dst-chunk (512 dense dst cols): batched indirect-DMA gather of
    h[src] rows -> msg tiles [128 edges, H] (edges on partitions)
  - per relation r: alpha-hot matrices [128 edges, 128 win] built by one DVE
    tensor_scalar (is_equal vs iota, scaled by 1/cnt); PE matmuls
    msg^T @ alphahot accumulate mean bins into PSUM [H, chunk]
  - transform: root matmul + 20 relation matmuls (W_r stationary, bf16)
    accumulate out^T [H, chunk] in PSUM; evacuation fuses BN partial stats
  - BatchNorm stats via AllReduce; affine+ReLU as one ACT op over [H, NS]
  - PE transposes h^T -> row-major shard -> DRAM -> AllGather
  - global mean pool via indicator matmuls + AllReduce; final linear+sigmoid
"""

import math
import sys

sys.path.insert(0, "/opt/trn_rl_repo")

import ml_dtypes
import numpy as np

import concourse.bacc as bacc
import concourse.bass as bass
import concourse.tile as tile
from concourse import mybir
from concourse import library_config
from concourse.bass_utils import run_bass_kernel_spmd

BF16 = ml_dtypes.bfloat16
P = 128


class Cfg:
    def __init__(self, N=100000, E=1600000, F=64, H=128, R=20, G=64, C=10, L=2,
                 NC=8, CHUNK=1024, WIN=128, EPS=1e-5, DT="bf16", DEBUG=False,
                 RSZ=25000):
        assert H == P
        self.N, self.E, self.F, self.H, self.R, self.G, self.C, self.L = (
            N, E, F, H, R, G, C, L)
        self.NC, self.CHUNK, self.WIN, self.EPS = NC, CHUNK, WIN, EPS
        self.DT = DT
        self.DEBUG = DEBUG
        self.RSZ = RSZ
        self.NREG = math.ceil(N / RSZ)
        assert N % NC == 0
        self.NS = N // NC
        self.nchunks = math.ceil(self.NS / CHUNK)
        self.cw = [min(CHUNK, self.NS - c * CHUNK) for c in range(self.nchunks)]
        self.nwin = [math.ceil(w / WIN) for w in self.cw]
        self.SENT = N  # sentinel gather index (> N-1 bound -> skipped)


def _plan(cfg, edge_index, edge_type, batch):
    """Host-side planner. Returns shared structure + per-core data arrays."""
    N, R, NC, NS, CHUNK, WIN = cfg.N, cfg.R, cfg.NC, cfg.NS, cfg.CHUNK, cfg.WIN
    src = edge_index[0].astype(np.int64)
    dst = edge_index[1].astype(np.int64)
    et = edge_type.astype(np.int64)

    comb = dst * R + et
    cnt = np.bincount(comb, minlength=N * R).astype(np.float64)
    alpha_e = (1.0 / np.maximum(cnt[comb], 1.0)).astype(np.float32)

    core = dst // NS
    dloc = dst % NS
    chunk = dloc // CHUNK
    inchunk = dloc % CHUNK
    win = inchunk // WIN
    loc = (inchunk % WIN).astype(np.float32)

    maxwin = max(cfg.nwin)
    gid = (chunk * R + et) * maxwin + win
    ngroups = cfg.nchunks * R * maxwin

    counts = np.zeros((NC, ngroups), np.int64)
    np.add.at(counts, (core, gid), 1)
    Tg = np.maximum(1, -(-counts.max(axis=0) // P))  # ceil div, min 1

    # tile order: chunk-major, then r, then win
    group_order = []
    for c in range(cfg.nchunks):
        for r in range(R):
            for w in range(cfg.nwin[c]):
                group_order.append((c * R + r) * maxwin + w)
    group_order = np.array(group_order, np.int64)
    tiles_of_group = Tg[group_order]
    tile_base = np.zeros(len(group_order), np.int64)
    np.cumsum(tiles_of_group[:-1], out=tile_base[1:])
    S_total = int(tiles_of_group.sum())

    gpos = np.full(ngroups, -1, np.int64)
    gpos[group_order] = np.arange(len(group_order))

    locA = np.full((NC, P, S_total), -1.0, np.float32)
    alphaA = np.zeros((NC, P, S_total), np.float32)
    srcA = np.zeros((NC, P, S_total), np.int64)  # src per slot (sentinel: -1)
    srcA[:] = -1

    # sort by src within each (core, gid) segment: ascending gather
    # addresses per slot-tile (better HBM locality); position within the
    # one-hot column comes from loc, so in-group order is free
    order = np.lexsort((src, gid, core))
    s_core, s_gid = core[order], gid[order]
    s_src, s_loc, s_alpha = src[order], loc[order], alpha_e[order]
    key = s_core * ngroups + s_gid
    first = np.r_[True, key[1:] != key[:-1]]
    grp_start = np.flatnonzero(first)
    seglen = np.diff(np.r_[grp_start, len(key)])
    rank = np.arange(len(key)) - np.repeat(grp_start, seglen)

    slot = tile_base[gpos[s_gid]] * P + rank
    srcA[s_core, slot % P, slot // P] = s_src
    locA[s_core, slot % P, slot // P] = s_loc
    alphaA[s_core, slot % P, slot // P] = s_alpha

    # emission structure: per chunk -> (slot_lo, slot_hi,
    #   per-r list of per-win (tile_base, ntiles))
    chunk_tiles = []
    for c in range(cfg.nchunks):
        lo = None
        hi = 0
        rlists = []
        for r in range(R):
            wl = []
            for w in range(cfg.nwin[c]):
                pos = gpos[(c * R + r) * maxwin + w]
                tb, tn = int(tile_base[pos]), int(tiles_of_group[pos])
                if lo is None:
                    lo = tb
                hi = tb + tn
                wl.append((tb, tn, w))
            rlists.append(wl)
        chunk_tiles.append((lo, hi, rlists))

    idxA = np.where(srcA >= 0, srcA, 0).astype(np.int32)

    gcnt = np.bincount(batch.astype(np.int64), minlength=cfg.G).astype(np.float32)
    inv_gcnt = 1.0 / np.maximum(gcnt, 1.0)

    return dict(S_total=S_total, chunk_tiles=chunk_tiles, locA=locA,
                alphaA=alphaA, inv_gcnt=inv_gcnt, idxA=idxA)


def _build_nc(cfg, plan):
    """Emit the SPMD Bass program (one program, NC cores)."""
    N, F, H, R, G, C, L = cfg.N, cfg.F, cfg.H, cfg.R, cfg.G, cfg.C, cfg.L
    NS, CHUNK, WIN = cfg.NS, cfg.CHUNK, cfg.WIN
    S_total = plan["S_total"]
    chunk_tiles = plan["chunk_tiles"]
    nblk = math.ceil(NS / P)

    nc = bacc.Bacc(None)
    f32, i32, i16 = mybir.dt.float32, mybir.dt.int32, mybir.dt.int16
    bf16 = mybir.dt.bfloat16 if cfg.DT == "bf16" else mybir.dt.float32
    AF = mybir.ActivationFunctionType
    OP = mybir.AluOpType

    xT_d = nc.dram_tensor("xT", [F, NS], f32, kind="ExternalInput")
    idx_d = nc.dram_tensor("idxA", [P, S_total], i32, kind="ExternalInput")
    iota_d = nc.dram_tensor("iotain", [P, WIN], bf16, kind="ExternalInput")
    identb_d = nc.dram_tensor("identb", [P, P], bf16, kind="ExternalInput")
    identf_d = nc.dram_tensor("identf", [P, P], f32, kind="ExternalInput")
    loc_d = nc.dram_tensor("locA", [P, S_total], f32, kind="ExternalInput")
    alp_d = nc.dram_tensor("alphaA", [P, S_total], f32, kind="ExternalInput")
    w_in_d = nc.dram_tensor("w_in", [F, H], f32, kind="ExternalInput")
    b_in_d = nc.dram_tensor("b_in", [H, 1], f32, kind="ExternalInput")
    relw_d = nc.dram_tensor("relw", [P, L * R + L, H], bf16, kind="ExternalInput")
    bng_d = nc.dram_tensor("bng", [H, L], f32, kind="ExternalInput")
    bnb_d = nc.dram_tensor("bnb", [H, L], f32, kind="ExternalInput")
    w_out_d = nc.dram_tensor("w_out", [H, C], f32, kind="ExternalInput")
    b_out_d = nc.dram_tensor("b_out", [C, 1], f32, kind="ExternalInput")
    gids_d = nc.dram_tensor("gids", [P, nblk], f32, kind="ExternalInput")
    invg_d = nc.dram_tensor("invg", [C, G], f32, kind="ExternalInput")
    out_d = nc.dram_tensor("out", [C, G], f32, kind="ExternalOutput")

    h_shard = [nc.dram_tensor(f"h_shard{l}", [NS, H], bf16) for l in range(L)]
    h_full = [nc.dram_tensor(f"h_full{l}", [N, H], bf16, addr_space="Shared")
              for l in range(L)]
    stats_in = nc.dram_tensor("stats_in", [H, 2], f32)
    stats_out = nc.dram_tensor("stats_out", [H, 2], f32, addr_space="Shared")
    pool_in = nc.dram_tensor("pool_in", [G, H], f32)
    pool_out = nc.dram_tensor("pool_out", [G, H], f32, addr_space="Shared")
    if cfg.DEBUG:
        dbg_h = [nc.dram_tensor(f"dbg_h{l}", [N, H], bf16,
                                kind="ExternalOutput") for l in range(L)]
        dbg_outb = nc.dram_tensor("dbg_outb", [L, H, NS], bf16,
                                  kind="ExternalOutput")
        dbg_stg = nc.dram_tensor("dbg_stg", [L, H, 8], f32,
                                 kind="ExternalOutput")
        dbg_pool = nc.dram_tensor("dbg_pool", [G, H], f32,
                                  kind="ExternalOutput")

    cores = list(range(cfg.NC))

    with tile.TileContext(nc) as tc:
        with (
            tc.tile_pool(name="const", bufs=1) as cpool,
            tc.tile_pool(name="big", bufs=1) as bigpool,
            tc.tile_pool(name="msg", bufs=32) as msgpool,
            tc.tile_pool(name="hot", bufs=16) as hotpool,
            tc.tile_pool(name="mean", bufs=1) as meanpool,
            tc.tile_pool(name="work", bufs=3) as workpool,
            tc.tile_pool(name="psA", bufs=2, space="PSUM") as psA,
            tc.tile_pool(name="psT", bufs=1, space="PSUM") as psT,
            tc.tile_pool(name="psB", bufs=1, space="PSUM") as psB,
        ):
            # ---------- constants ----------
            iota_bf = cpool.tile([P, WIN], bf16, tag="iota_bf")
            nc.sync.dma_start(iota_bf[:], iota_d[:])
            ident = cpool.tile([P, P], bf16, tag="ident")
            nc.sync.dma_start(ident[:], identb_d[:])
            identf = cpool.tile([P, P], f32, tag="identf")
            nc.sync.dma_start(identf[:], identf_d[:])

            idx_t = cpool.tile([P, S_total], i32, tag="idx")
            nc.sync.dma_start(idx_t[:], idx_d[:])
            loc_t = cpool.tile([P, S_total], f32, tag="loc")
            nc.sync.dma_start(loc_t[:], loc_d[:])
            alp_t = cpool.tile([P, S_total], f32, tag="alp")
            nc.sync.dma_start(alp_t[:], alp_d[:])

            relw_t = cpool.tile([P, L * R + L, H], bf16, tag="relw")
            nc.sync.dma_start(relw_t[:], relw_d[:])
            w_in_t = cpool.tile([F, H], f32, tag="w_in")
            nc.sync.dma_start(w_in_t[:], w_in_d[:])
            b_in_t = cpool.tile([H, 1], f32, tag="b_in")
            nc.sync.dma_start(b_in_t[:], b_in_d[:])
            bng_t = cpool.tile([H, L], f32, tag="bng")
            nc.sync.dma_start(bng_t[:], bng_d[:])
            bnb_t = cpool.tile([H, L], f32, tag="bnb")
            nc.sync.dma_start(bnb_t[:], bnb_d[:])
            gids_t = cpool.tile([P, nblk], f32, tag="gids")
            nc.sync.dma_start(gids_t[:], gids_d[:])
            w_out_t = cpool.tile([H, C], f32, tag="w_out")
            nc.sync.dma_start(w_out_t[:], w_out_d[:])
            b_out_t = cpool.tile([C, 1], f32, tag="b_out")
            nc.sync.dma_start(b_out_t[:], b_out_d[:])
            invg_t = cpool.tile([C, G], f32, tag="invg")
            nc.sync.dma_start(invg_t[:], invg_d[:])

            # fences: pull const-load DMA completions into engine program
            # order one DMA at a time, so compute ops (tiny ISA wait
            # budgets) emit no DMA waits of their own
            fence = cpool.tile([1, 1], f32, tag="fence")
            for _ft in (gids_t, bng_t, bnb_t, invg_t, w_in_t, w_out_t):
                nc.vector.tensor_copy(fence[:], _ft[0:1, 0:1].bitcast(f32))
            for _ft in (loc_t, alp_t):
                nc.vector.tensor_copy(fence[:], _ft[0:1, 0:1].bitcast(f32))
            fenceA = cpool.tile([1, 1], f32, tag="fenceA")
            for _ft in (b_in_t, b_out_t):
                nc.scalar.copy(fenceA[:], _ft[0:1, 0:1])
            _rw = 2 if cfg.DT == "bf16" else 1
            nc.scalar.copy(fenceA[:], relw_t[0:1, 0, 0:_rw].bitcast(f32))

            hT = bigpool.tile([P, NS], bf16, tag="hT")
            outb = bigpool.tile([P, NS], bf16, tag="outb")
            sum_parts = bigpool.tile([P, cfg.nchunks], f32, tag="sumP")
            sq_parts = bigpool.tile([P, cfg.nchunks], f32, tag="sqP")
            sq_scr = bigpool.tile([P, CHUNK], bf16, tag="sqscr")

            # ---------- input MLP ----------
            for c in range(cfg.nchunks):
                cw = cfg.cw[c]
                xc = workpool.tile([F, CHUNK], f32, tag="xc")
                nc.sync.dma_start(xc[:, :cw], xT_d[:, c * CHUNK:c * CHUNK + cw])
                ps = psB.tile([P, CHUNK], f32, tag="psB")
                for h0 in range(0, cw, 512):
                    hw_ = min(512, cw - h0)
                    nc.tensor.matmul(out=ps[:, h0:h0 + hw_], lhsT=w_in_t[:],
                                     rhs=xc[:, h0:h0 + hw_],
                                     start=True, stop=True)
                nc.scalar.activation(hT[:, c * CHUNK:c * CHUNK + cw], ps[:, :cw],
                                     AF.Relu, bias=b_in_t[:, 0:1], scale=1.0)

            def emit_transpose_store(l):
                for b in range(nblk):
                    bw = min(P, NS - b * P)
                    pst = psT.tile([P, P], bf16, tag="psT")
                    nc.tensor.transpose(pst[:bw, :P], hT[:, b * P:b * P + bw],
                                        ident[:])
                    rm = workpool.tile([P, P], bf16, tag="rm")
                    nc.vector.tensor_copy(rm[:bw, :], pst[:bw, :P])
                    nc.sync.dma_start(h_shard[l][b * P:b * P + bw, :], rm[:bw, :])
                nc.gpsimd.collective_compute(
                    "AllGather", OP.bypass, replica_groups=[cores],
                    ins=[h_shard[l][:]], outs=[h_full[l][:]])
                if cfg.DEBUG:
                    nc.gpsimd.dma_start(dbg_h[l][:], h_full[l][:])

            emit_transpose_store(0)

            # ---------- RGCN layers ----------
            for l in range(L):
                root_i = L * R + l
                for c in range(cfg.nchunks):
                    cw = cfg.cw[c]
                    lo, hi, rlists = chunk_tiles[c]
                    mean = meanpool.tile([P, R, CHUNK], bf16, tag="mean")
                    for r in range(R):
                        psa = psA.tile([P, CHUNK], f32, tag="psA")
                        for (tb, tn, w) in rlists[r]:
                            ww = min(WIN, cw - w * WIN)
                            for t in range(tn):
                                s = tb + t
                                m = msgpool.tile([P, H], bf16, tag="msg")
                                nc.gpsimd.indirect_dma_start(
                                    out=m[:], out_offset=None,
                                    in_=h_full[l][:],
                                    in_offset=bass.IndirectOffsetOnAxis(
                                        ap=idx_t[:, s:s + 1], axis=0))
                                hot = hotpool.tile([P, WIN], bf16, tag="hot")
                                nc.vector.tensor_scalar(
                                    out=hot[:, :ww], in0=iota_bf[:, :ww],
                                    scalar1=loc_t[:, s:s + 1],
                                    scalar2=alp_t[:, s:s + 1],
                                    op0=OP.is_equal, op1=OP.mult)
                                nc.tensor.matmul(
                                    out=psa[:, w * WIN:w * WIN + ww],
                                    lhsT=m[:], rhs=hot[:, :ww],
                                    start=(t == 0), stop=(t == tn - 1))
                        if r % 2 == 0:
                            nc.vector.tensor_copy(mean[:, r, :cw], psa[:, :cw])
                        else:
                            nc.scalar.copy(mean[:, r, :cw], psa[:, :cw])

                    psb = psB.tile([P, CHUNK], f32, tag="psB")
                    # split transform at the 512-col PSUM bank boundary
                    for h0 in range(0, cw, 512):
                        hw_ = min(512, cw - h0)
                        nc.tensor.matmul(
                            out=psb[:, h0:h0 + hw_], lhsT=relw_t[:, root_i, :],
                            rhs=hT[:, c * CHUNK + h0:c * CHUNK + h0 + hw_],
                            start=True, stop=False)
                        for r in range(R):
                            nc.tensor.matmul(out=psb[:, h0:h0 + hw_],
                                             lhsT=relw_t[:, l * R + r, :],
                                             rhs=mean[:, r, h0:h0 + hw_],
                                             start=False, stop=(r == R - 1))

                    nc.vector.tensor_scalar(
                        out=outb[:, c * CHUNK:c * CHUNK + cw], in0=psb[:, :cw],
                        scalar1=1.0, scalar2=None, op0=OP.mult, op1=OP.add,
                        accum_out=sum_parts[:, c:c + 1])
                    nc.scalar.activation(sq_scr[:, :cw], psb[:, :cw], AF.Square,
                                         accum_out=sq_parts[:, c:c + 1])

                # ---------- BatchNorm + ReLU ----------
                st = workpool.tile([H, 2], f32, tag="stats")
                nc.vector.reduce_sum(st[:, 0:1], sum_parts[:],
                                     axis=mybir.AxisListType.X)
                nc.vector.reduce_sum(st[:, 1:2], sq_parts[:],
                                     axis=mybir.AxisListType.X)
                nc.sync.dma_start(stats_in[:], st[:])
                nc.gpsimd.collective_compute(
                    "AllReduce", OP.add, replica_groups=[cores],
                    ins=[stats_in[:]], outs=[stats_out[:]])
                stg = workpool.tile([H, 8], f32, tag="stg")
                nc.sync.dma_start(stg[:, 0:2], stats_out[:])
                nc.vector.tensor_scalar(out=stg[:, 2:3], in0=stg[:, 0:1],
                                        scalar1=1.0 / N, scalar2=None,
                                        op0=OP.mult)
                nc.vector.tensor_scalar(out=stg[:, 3:4], in0=stg[:, 1:2],
                                        scalar1=1.0 / N, scalar2=None,
                                        op0=OP.mult)
                nc.vector.tensor_tensor(out=stg[:, 4:5], in0=stg[:, 2:3],
                                        in1=stg[:, 2:3], op=OP.mult)
                nc.vector.tensor_tensor(out=stg[:, 4:5], in0=stg[:, 3:4],
                                        in1=stg[:, 4:5], op=OP.subtract)
                nc.vector.tensor_scalar(out=stg[:, 4:5], in0=stg[:, 4:5],
                                        scalar1=cfg.EPS, scalar2=None,
                                        op0=OP.add)
                nc.scalar.sqrt(stg[:, 5:6], stg[:, 4:5])
                nc.vector.reciprocal(stg[:, 6:7], stg[:, 5:6])
                nc.vector.tensor_tensor(out=stg[:, 6:7], in0=stg[:, 6:7],
                                        in1=bng_t[:, l:l + 1], op=OP.mult)
                nc.vector.tensor_tensor(out=stg[:, 7:8], in0=stg[:, 6:7],
                                        in1=stg[:, 2:3], op=OP.mult)
                nc.vector.tensor_tensor(out=stg[:, 7:8], in0=bnb_t[:, l:l + 1],
                                        in1=stg[:, 7:8], op=OP.subtract)
                if cfg.DEBUG:
                    nc.sync.dma_start(dbg_outb[l], outb[:])
                    nc.sync.dma_start(dbg_stg[l], stg[:])
                nc.scalar.activation(hT[:], outb[:], AF.Relu,
                                     bias=stg[:, 7:8], scale=stg[:, 6:7])

                if l + 1 < L:
                    emit_transpose_store(l + 1)

            # ---------- global mean pool + output MLP ----------
            psp = psB.tile([G, CHUNK], f32, tag="psB")
            for b in range(nblk):
                bw = min(P, NS - b * P)
                pst = psT.tile([P, P], bf16, tag="psT")
                nc.tensor.transpose(pst[:bw, :P], hT[:, b * P:b * P + bw],
                                    ident[:])
                rm = workpool.tile([P, P], bf16, tag="rm")
                nc.vector.tensor_copy(rm[:bw, :], pst[:bw, :P])
                ind = hotpool.tile([P, G], bf16, tag="ind")
                nc.vector.tensor_scalar(out=ind[:bw, :], in0=iota_bf[:bw, :G],
                                        scalar1=gids_t[:bw, b:b + 1],
                                        scalar2=None, op0=OP.is_equal)
                nc.tensor.matmul(out=psp[:, :H], lhsT=ind[:bw, :],
                                 rhs=rm[:bw, :], start=(b == 0),
                                 stop=(b == nblk - 1))
            poolt = workpool.tile([G, H], f32, tag="poolt")
            nc.vector.tensor_copy(poolt[:], psp[:, :H])
            nc.sync.dma_start(pool_in[:], poolt[:])
            nc.gpsimd.collective_compute(
                "AllReduce", OP.add, replica_groups=[cores],
                ins=[pool_in[:]], outs=[pool_out[:]])
            poolg = workpool.tile([G, H], f32, tag="poolg")
            nc.sync.dma_start(poolg[:], pool_out[:])
            if cfg.DEBUG:
                nc.gpsimd.dma_start(dbg_pool[:], pool_out[:])

            pstT = psT.tile([P, G], f32, tag="psTf")
            nc.tensor.transpose(pstT[:, :G], poolg[:], identf[:G, :G])
            poolT = workpool.tile([P, G], f32, tag="poolT")
            nc.vector.tensor_copy(poolT[:], pstT[:, :G])

            psl = psB.tile([C, CHUNK], f32, tag="psB")
            nc.tensor.matmul(out=psl[:, :G], lhsT=w_out_t[:], rhs=poolT[:],
                             start=True, stop=True)
            logit = workpool.tile([C, G], f32, tag="logit")
            nc.vector.tensor_tensor(out=logit[:], in0=psl[:, :G], in1=invg_t[:],
                                    op=OP.mult)
            logit2 = workpool.tile([C, G], f32, tag="logit2")
            nc.scalar.activation(logit2[:], logit[:], AF.Sigmoid,
                                 bias=b_out_t[:, 0:1], scale=1.0)
            nc.sync.dma_start(out_d[:], logit2[:])

    return nc


def _make_in_maps(cfg, plan, inputs):
    H, C, G, F, NS, R, L = cfg.H, cfg.C, cfg.G, cfg.F, cfg.NS, cfg.R, cfg.L
    x = np.asarray(inputs["x"], np.float32)
    batch = np.asarray(inputs["batch"])

    np_dt = BF16 if cfg.DT == "bf16" else np.float32
    relw = np.empty((L * R + L, P, H), np_dt)
    rel_w = np.asarray(inputs["rel_w"], np.float32)
    root_w = np.asarray(inputs["root_w"], np.float32)
    for l in range(L):
        for r in range(R):
            relw[l * R + r] = rel_w[l, r].astype(np_dt)
        relw[L * R + l] = root_w[l].astype(np_dt)

    bng = np.ascontiguousarray(np.asarray(inputs["bn_g"], np.float32).T)
    bnb = np.ascontiguousarray(np.asarray(inputs["bn_b"], np.float32).T)
    b_in = np.asarray(inputs["b_in"], np.float32).reshape(H, 1)
    b_out = np.asarray(inputs["b_out"], np.float32).reshape(C, 1)
    w_in = np.asarray(inputs["w_in"], np.float32)
    w_out = np.asarray(inputs["w_out"], np.float32)
    invg = np.ascontiguousarray(
        np.broadcast_to(plan["inv_gcnt"][None, :], (C, G)))
    relw_p = np.ascontiguousarray(relw.transpose(1, 0, 2))  # [P, 42, H]
    iota_np = np.broadcast_to(np.arange(cfg.WIN, dtype=np.float32)[None, :],
                              (P, cfg.WIN)).astype(BF16).copy()
    ident_np = np.eye(P, dtype=np.float32).astype(BF16)
    identf_np = np.eye(P, dtype=np.float32)

    nblk = math.ceil(NS / P)
    in_maps = []
    for c in range(cfg.NC):
        lo, hi = c * NS, (c + 1) * NS
        xT = np.ascontiguousarray(x[lo:hi].T)
        gids = np.full((P, nblk), -1.0, np.float32)
        bseg = batch[lo:hi].astype(np.float32)
        for b in range(nblk):
            bw = min(P, NS - b * P)
            gids[:bw, b] = bseg[b * P:b * P + bw]
        in_maps.append(dict(
            xT=xT, idxA=np.ascontiguousarray(plan["idxA"][c]),
            iotain=iota_np, identb=ident_np, identf=identf_np,
            locA=np.ascontiguousarray(plan["locA"][c]),
            alphaA=np.ascontiguousarray(plan["alphaA"][c]),
            w_in=w_in, b_in=b_in, relw=relw_p, bng=bng, bnb=bnb,
            w_out=w_out, b_out=b_out, gids=gids, invg=invg,
        ))
    return in_maps


def _plan_key(cfg, plan):
    def _freeze(x):
        if isinstance(x, (list, tuple)):
            return tuple(_freeze(v) for v in x)
        return x
    return (cfg.N, cfg.E, cfg.F, cfg.H, cfg.R, cfg.G, cfg.C, cfg.L, cfg.NC,
            cfg.CHUNK, cfg.WIN, cfg.DT, cfg.DEBUG, plan["S_total"],
            _freeze(plan["chunk_tiles"]))


class _Runner:
    """Compile-once PJRT runner (mirrors run_bass_kernel_spmd's axon path,
    but keeps the jitted callable so repeat invocations skip retrace,
    XLA recompile, and executable reload)."""

    def __init__(self, cfg, plan):
        import jax
        from jax.sharding import Mesh, PartitionSpec, NamedSharding
        try:
            from jax.experimental.shard_map import shard_map
        except ImportError:
            from jax import shard_map
        from concourse.bass2jax import (
            _bass_exec_p, partition_id_tensor, install_neuronx_cc_hook)

        self.cfg = cfg
        nc = _build_nc(cfg, plan)
        if not nc.is_finalized():
            nc.finalize()
        self.nc = nc
        install_neuronx_cc_hook()
        assert nc.dbg_addr is None

        pname = nc.partition_id_tensor.name if nc.partition_id_tensor else None
        in_names, out_names, out_avals, self.zero_shapes = [], [], [], []
        for alloc in nc.m.functions[0].allocations:
            if not isinstance(alloc, mybir.MemoryLocationSet):
                continue
            name = alloc.memorylocations[0].name
            if alloc.kind == "ExternalInput":
                if name != pname:
                    in_names.append(name)
            elif alloc.kind == "ExternalOutput":
                out_names.append(name)
                shape = tuple(alloc.tensor_shape)
                dtype = mybir.dt.np(alloc.dtype)
                out_avals.append(jax.core.ShapedArray(shape, dtype))
                self.zero_shapes.append((shape, dtype))
        self.in_names, self.out_names = in_names, out_names
        self.out_avals = out_avals
        n_params, n_outs = len(in_names), len(out_avals)
        all_names = in_names + out_names + ([pname] if pname else [])
        donate = tuple(range(n_params, n_params + n_outs))

        def _body(*args):
            operands = list(args)
            if pname is not None:
                operands.append(partition_id_tensor())
            return tuple(_bass_exec_p.bind(
                *operands, out_avals=tuple(out_avals),
                in_names=tuple(all_names), out_names=tuple(out_names),
                lowering_input_output_aliases=(),
                sim_require_finite=True, sim_require_nnan=True, nc=nc))

        devices = jax.devices()[:cfg.NC]
        assert len(devices) == cfg.NC
        self.mesh = Mesh(np.asarray(devices), ("core",))
        self.shard = NamedSharding(self.mesh, PartitionSpec("core"))
        self.jit = jax.jit(
            shard_map(_body, mesh=self.mesh,
                      in_specs=(PartitionSpec("core"),) * (n_params + n_outs),
                      out_specs=(PartitionSpec("core"),) * n_outs,
                      check_rep=False),
            donate_argnums=donate, keep_unused=True)

    def concat_inputs(self, in_maps):
        return [np.concatenate([np.asarray(m[n]) for m in in_maps], axis=0)
                for n in self.in_names]

    def put_inputs(self, concat):
        import jax
        dev = [jax.device_put(a, self.shard) for a in concat]
        jax.block_until_ready(dev)
        return dev

    def zeros(self, device=False):
        import jax
        zs = [np.zeros((self.cfg.NC * s[0], *s[1:]), dt)
              for (s, dt) in self.zero_shapes]
        if device:
            zs = [jax.device_put(z, self.shard) for z in zs]
            jax.block_until_ready(zs)
        return zs

    def __call__(self, ins, zeros):
        return self.jit(*ins, *zeros)

    def out_core0(self, outs):
        i = self.out_names.index("out")
        shape = self.out_avals[i].shape
        return np.asarray(outs[i]).reshape(self.cfg.NC, *shape)[0]


_RUNNER_CACHE = {}


def _get_runner(cfg, plan):
    key = _plan_key(cfg, plan)
    r = _RUNNER_CACHE.get(key)
    if r is None:
        r = _Runner(cfg, plan)
        _RUNNER_CACHE[key] = r
    return r


def _run(cfg, inputs, **kw):
    plan = _plan(cfg, np.asarray(inputs["edge_index"]),
                 np.asarray(inputs["edge_type"]), np.asarray(inputs["batch"]))
    runner = _get_runner(cfg, plan)
    in_maps = _make_in_maps(cfg, plan, inputs)
    concat = runner.concat_inputs(in_maps)
    outs = runner(concat, runner.zeros())
    out = runner.out_core0(outs)
    return np.ascontiguousarray(np.asarray(out).T.astype(np.float32)), runner


def kernel(**inputs):
    cfg = Cfg()
    out, _ = _run(cfg, inputs)
    return out



# revision 16
# speedup vs baseline: 307.0391x; 1.0365x over previous
"""Trainium2 Bass kernel for 2-layer RGCN (nn_PygModel_52003464020165).

Self-contained: accepts FULL inputs, shards across 8 NeuronCores internally,
returns FULL [64, 10] output.

Architecture (per core, dst-sharded graph):
  - full h replicated each layer via AllGather (bf16, [N, H] row-major in DRAM)
  - per dst-chunk (512 dense dst cols): batched indirect-DMA gather of
    h[src] rows -> msg tiles [128 edges, H] (edges on partitions)
  - per relation r: alpha-hot matrices [128 edges, 128 win] built by one DVE
    tensor_scalar (is_equal vs iota, scaled by 1/cnt); PE matmuls
    msg^T @ alphahot accumulate mean bins into PSUM [H, chunk]
  - transform: root matmul + 20 relation matmuls (W_r stationary, bf16)
    accumulate out^T [H, chunk] in PSUM; evacuation fuses BN partial stats
  - BatchNorm stats via AllReduce; affine+ReLU as one ACT op over [H, NS]
  - PE transposes h^T -> row-major shard -> DRAM -> AllGather
  - global mean pool via indicator matmuls + AllReduce; final linear+sigmoid
"""

import math
import sys

sys.path.insert(0, "/opt/trn_rl_repo")

import ml_dtypes
import numpy as np

import concourse.bacc as bacc
import concourse.bass as bass
import concourse.tile as tile
from concourse import mybir
from concourse import library_config
from concourse.bass_utils import run_bass_kernel_spmd

BF16 = ml_dtypes.bfloat16
P = 128


class Cfg:
    def __init__(self, N=100000, E=1600000, F=64, H=128, R=20, G=64, C=10, L=2,
                 NC=8, CHUNK=1024, WIN=128, EPS=1e-5, DT="bf16", DEBUG=False,
                 RSZ=25000):
        assert H == P
        self.N, self.E, self.F, self.H, self.R, self.G, self.C, self.L = (
            N, E, F, H, R, G, C, L)
        self.NC, self.CHUNK, self.WIN, self.EPS = NC, CHUNK, WIN, EPS
        self.DT = DT
        self.DEBUG = DEBUG
        self.RSZ = RSZ
        self.NREG = math.ceil(N / RSZ)
        assert N % NC == 0
        self.NS = N // NC
        self.nchunks = math.ceil(self.NS / CHUNK)
        self.cw = [min(CHUNK, self.NS - c * CHUNK) for c in range(self.nchunks)]
        self.nwin = [math.ceil(w / WIN) for w in self.cw]
        self.SENT = N  # sentinel gather index (> N-1 bound -> skipped)


def _plan(cfg, edge_index, edge_type, batch):
    """Host-side planner. Returns shared structure + per-core data arrays."""
    N, R, NC, NS, CHUNK, WIN = cfg.N, cfg.R, cfg.NC, cfg.NS, cfg.CHUNK, cfg.WIN
    src = edge_index[0].astype(np.int64)
    dst = edge_index[1].astype(np.int64)
    et = edge_type.astype(np.int64)

    comb = dst * R + et
    cnt = np.bincount(comb, minlength=N * R).astype(np.float64)
    alpha_e = (1.0 / np.maximum(cnt[comb], 1.0)).astype(np.float32)

    core = dst // NS
    dloc = dst % NS
    chunk = dloc // CHUNK
    inchunk = dloc % CHUNK
    win = inchunk // WIN
    loc = (inchunk % WIN).astype(np.float32)

    maxwin = max(cfg.nwin)
    gid = (chunk * R + et) * maxwin + win
    ngroups = cfg.nchunks * R * maxwin

    counts = np.zeros((NC, ngroups), np.int64)
    np.add.at(counts, (core, gid), 1)
    Tg = np.maximum(1, -(-counts.max(axis=0) // P))  # ceil div, min 1

    # tile order: chunk-major, then r, then win
    group_order = []
    for c in range(cfg.nchunks):
        for r in range(R):
            for w in range(cfg.nwin[c]):
                group_order.append((c * R + r) * maxwin + w)
    group_order = np.array(group_order, np.int64)
    tiles_of_group = Tg[group_order]
    tile_base = np.zeros(len(group_order), np.int64)
    np.cumsum(tiles_of_group[:-1], out=tile_base[1:])
    S_total = int(tiles_of_group.sum())

    gpos = np.full(ngroups, -1, np.int64)
    gpos[group_order] = np.arange(len(group_order))

    locA = np.full((NC, P, S_total), -1.0, np.float32)
    alphaA = np.zeros((NC, P, S_total), np.float32)
    srcA = np.zeros((NC, P, S_total), np.int64)  # src per slot (sentinel: -1)
    srcA[:] = -1

    # sort by src within each (core, gid) segment: ascending gather
    # addresses per slot-tile (better HBM locality); position within the
    # one-hot column comes from loc, so in-group order is free
    order = np.lexsort((src, gid, core))
    s_core, s_gid = core[order], gid[order]
    s_src, s_loc, s_alpha = src[order], loc[order], alpha_e[order]
    key = s_core * ngroups + s_gid
    first = np.r_[True, key[1:] != key[:-1]]
    grp_start = np.flatnonzero(first)
    seglen = np.diff(np.r_[grp_start, len(key)])
    rank = np.arange(len(key)) - np.repeat(grp_start, seglen)

    slot = tile_base[gpos[s_gid]] * P + rank
    srcA[s_core, slot % P, slot // P] = s_src
    locA[s_core, slot % P, slot // P] = s_loc
    alphaA[s_core, slot % P, slot // P] = s_alpha

    # emission structure: per chunk -> (slot_lo, slot_hi,
    #   per-r list of per-win (tile_base, ntiles))
    chunk_tiles = []
    for c in range(cfg.nchunks):
        lo = None
        hi = 0
        rlists = []
        for r in range(R):
            wl = []
            for w in range(cfg.nwin[c]):
                pos = gpos[(c * R + r) * maxwin + w]
                tb, tn = int(tile_base[pos]), int(tiles_of_group[pos])
                if lo is None:
                    lo = tb
                hi = tb + tn
                wl.append((tb, tn, w))
            rlists.append(wl)
        chunk_tiles.append((lo, hi, rlists))

    idxA = np.where(srcA >= 0, srcA, 0).astype(np.int32)

    gcnt = np.bincount(batch.astype(np.int64), minlength=cfg.G).astype(np.float32)
    inv_gcnt = 1.0 / np.maximum(gcnt, 1.0)

    return dict(S_total=S_total, chunk_tiles=chunk_tiles, locA=locA,
                alphaA=alphaA, inv_gcnt=inv_gcnt, idxA=idxA)


def _build_nc(cfg, plan):
    """Emit the SPMD Bass program (one program, NC cores)."""
    N, F, H, R, G, C, L = cfg.N, cfg.F, cfg.H, cfg.R, cfg.G, cfg.C, cfg.L
    NS, CHUNK, WIN = cfg.NS, cfg.CHUNK, cfg.WIN
    S_total = plan["S_total"]
    chunk_tiles = plan["chunk_tiles"]
    nblk = math.ceil(NS / P)

    nc = bacc.Bacc(None)
    f32, i32, i16 = mybir.dt.float32, mybir.dt.int32, mybir.dt.int16
    bf16 = mybir.dt.bfloat16 if cfg.DT == "bf16" else mybir.dt.float32
    AF = mybir.ActivationFunctionType
    OP = mybir.AluOpType

    xT_d = nc.dram_tensor("xT", [F, NS], f32, kind="ExternalInput")
    idx_d = nc.dram_tensor("idxA", [P, S_total], i32, kind="ExternalInput")
    iota_d = nc.dram_tensor("iotain", [P, WIN], bf16, kind="ExternalInput")
    identb_d = nc.dram_tensor("identb", [P, P], bf16, kind="ExternalInput")
    identf_d = nc.dram_tensor("identf", [P, P], f32, kind="ExternalInput")
    loc_d = nc.dram_tensor("locA", [P, S_total], f32, kind="ExternalInput")
    alp_d = nc.dram_tensor("alphaA", [P, S_total], f32, kind="ExternalInput")
    w_in_d = nc.dram_tensor("w_in", [F, H], f32, kind="ExternalInput")
    b_in_d = nc.dram_tensor("b_in", [H, 1], f32, kind="ExternalInput")
    relw_d = nc.dram_tensor("relw", [P, L * R + L, H], bf16, kind="ExternalInput")
    bng_d = nc.dram_tensor("bng", [H, L], f32, kind="ExternalInput")
    bnb_d = nc.dram_tensor("bnb", [H, L], f32, kind="ExternalInput")
    w_out_d = nc.dram_tensor("w_out", [H, C], f32, kind="ExternalInput")
    b_out_d = nc.dram_tensor("b_out", [C, 1], f32, kind="ExternalInput")
    gids_d = nc.dram_tensor("gids", [P, nblk], f32, kind="ExternalInput")
    invg_d = nc.dram_tensor("invg", [C, G], f32, kind="ExternalInput")
    out_d = nc.dram_tensor("out", [C, G], f32, kind="ExternalOutput")

    h_shard = [nc.dram_tensor(f"h_shard{l}", [NS, H], bf16) for l in range(L)]
    h_full = [nc.dram_tensor(f"h_full{l}", [N, H], bf16, addr_space="Shared")
              for l in range(L)]
    stats_in = nc.dram_tensor("stats_in", [H, 2], f32)
    stats_out = nc.dram_tensor("stats_out", [H, 2], f32, addr_space="Shared")
    pool_in = nc.dram_tensor("pool_in", [G, H], f32)
    pool_out = nc.dram_tensor("pool_out", [G, H], f32, addr_space="Shared")
    if cfg.DEBUG:
        dbg_h = [nc.dram_tensor(f"dbg_h{l}", [N, H], bf16,
                                kind="ExternalOutput") for l in range(L)]
        dbg_outb = nc.dram_tensor("dbg_outb", [L, H, NS], bf16,
                                  kind="ExternalOutput")
        dbg_stg = nc.dram_tensor("dbg_stg", [L, H, 8], f32,
                                 kind="ExternalOutput")
        dbg_pool = nc.dram_tensor("dbg_pool", [G, H], f32,
                                  kind="ExternalOutput")

    cores = list(range(cfg.NC))

    with tile.TileContext(nc) as tc:
        with (
            tc.tile_pool(name="const", bufs=1) as cpool,
            tc.tile_pool(name="big", bufs=1) as bigpool,
            tc.tile_pool(name="msg", bufs=64) as msgpool,
            tc.tile_pool(name="hot", bufs=32) as hotpool,
            tc.tile_pool(name="mean", bufs=1) as meanpool,
            tc.tile_pool(name="work", bufs=3) as workpool,
            tc.tile_pool(name="psA", bufs=2, space="PSUM") as psA,
            tc.tile_pool(name="psT", bufs=1, space="PSUM") as psT,
            tc.tile_pool(name="psB", bufs=1, space="PSUM") as psB,
        ):
            # ---------- constants ----------
            iota_bf = cpool.tile([P, WIN], bf16, tag="iota_bf")
            nc.sync.dma_start(iota_bf[:], iota_d[:])
            ident = cpool.tile([P, P], bf16, tag="ident")
            nc.sync.dma_start(ident[:], identb_d[:])
            identf = cpool.tile([P, P], f32, tag="identf")
            nc.sync.dma_start(identf[:], identf_d[:])

            idx_t = cpool.tile([P, S_total], i32, tag="idx")
            nc.sync.dma_start(idx_t[:], idx_d[:])
            loc_t = cpool.tile([P, S_total], f32, tag="loc")
            nc.sync.dma_start(loc_t[:], loc_d[:])
            alp_t = cpool.tile([P, S_total], f32, tag="alp")
            nc.sync.dma_start(alp_t[:], alp_d[:])

            relw_t = cpool.tile([P, L * R + L, H], bf16, tag="relw")
            nc.sync.dma_start(relw_t[:], relw_d[:])
            w_in_t = cpool.tile([F, H], f32, tag="w_in")
            nc.sync.dma_start(w_in_t[:], w_in_d[:])
            b_in_t = cpool.tile([H, 1], f32, tag="b_in")
            nc.sync.dma_start(b_in_t[:], b_in_d[:])
            bng_t = cpool.tile([H, L], f32, tag="bng")
            nc.sync.dma_start(bng_t[:], bng_d[:])
            bnb_t = cpool.tile([H, L], f32, tag="bnb")
            nc.sync.dma_start(bnb_t[:], bnb_d[:])
            gids_t = cpool.tile([P, nblk], f32, tag="gids")
            nc.sync.dma_start(gids_t[:], gids_d[:])
            w_out_t = cpool.tile([H, C], f32, tag="w_out")
            nc.sync.dma_start(w_out_t[:], w_out_d[:])
            b_out_t = cpool.tile([C, 1], f32, tag="b_out")
            nc.sync.dma_start(b_out_t[:], b_out_d[:])
            invg_t = cpool.tile([C, G], f32, tag="invg")
            nc.sync.dma_start(invg_t[:], invg_d[:])

            # fences: pull const-load DMA completions into engine program
            # order one DMA at a time, so compute ops (tiny ISA wait
            # budgets) emit no DMA waits of their own
            fence = cpool.tile([1, 1], f32, tag="fence")
            for _ft in (gids_t, bng_t, bnb_t, invg_t, w_in_t, w_out_t):
                nc.vector.tensor_copy(fence[:], _ft[0:1, 0:1].bitcast(f32))
            for _ft in (loc_t, alp_t):
                nc.vector.tensor_copy(fence[:], _ft[0:1, 0:1].bitcast(f32))
            fenceA = cpool.tile([1, 1], f32, tag="fenceA")
            for _ft in (b_in_t, b_out_t):
                nc.scalar.copy(fenceA[:], _ft[0:1, 0:1])
            _rw = 2 if cfg.DT == "bf16" else 1
            nc.scalar.copy(fenceA[:], relw_t[0:1, 0, 0:_rw].bitcast(f32))

            hT = bigpool.tile([P, NS], bf16, tag="hT")
            outb = bigpool.tile([P, NS], bf16, tag="outb")
            sum_parts = bigpool.tile([P, cfg.nchunks], f32, tag="sumP")
            sq_parts = bigpool.tile([P, cfg.nchunks], f32, tag="sqP")
            sq_scr = bigpool.tile([P, CHUNK], bf16, tag="sqscr")

            # ---------- input MLP ----------
            for c in range(cfg.nchunks):
                cw = cfg.cw[c]
                xc = workpool.tile([F, CHUNK], f32, tag="xc")
                nc.sync.dma_start(xc[:, :cw], xT_d[:, c * CHUNK:c * CHUNK + cw])
                ps = psB.tile([P, CHUNK], f32, tag="psB")
                for h0 in range(0, cw, 512):
                    hw_ = min(512, cw - h0)
                    nc.tensor.matmul(out=ps[:, h0:h0 + hw_], lhsT=w_in_t[:],
                                     rhs=xc[:, h0:h0 + hw_],
                                     start=True, stop=True)
                nc.scalar.activation(hT[:, c * CHUNK:c * CHUNK + cw], ps[:, :cw],
                                     AF.Relu, bias=b_in_t[:, 0:1], scale=1.0)

            def emit_transpose_store(l):
                for b in range(nblk):
                    bw = min(P, NS - b * P)
                    pst = psT.tile([P, P], bf16, tag="psT")
                    nc.tensor.transpose(pst[:bw, :P], hT[:, b * P:b * P + bw],
                                        ident[:])
                    rm = workpool.tile([P, P], bf16, tag="rm")
                    nc.vector.tensor_copy(rm[:bw, :], pst[:bw, :P])
                    nc.sync.dma_start(h_shard[l][b * P:b * P + bw, :], rm[:bw, :])
                nc.gpsimd.collective_compute(
                    "AllGather", OP.bypass, replica_groups=[cores],
                    ins=[h_shard[l][:]], outs=[h_full[l][:]])
                if cfg.DEBUG:
                    nc.gpsimd.dma_start(dbg_h[l][:], h_full[l][:])

            emit_transpose_store(0)

            # ---------- RGCN layers ----------
            for l in range(L):
                root_i = L * R + l
                for c in range(cfg.nchunks):
                    cw = cfg.cw[c]
                    lo, hi, rlists = chunk_tiles[c]
                    mean = meanpool.tile([P, R, CHUNK], bf16, tag="mean")
                    for r in range(R):
                        psa = psA.tile([P, CHUNK], f32, tag="psA")
                        for (tb, tn, w) in rlists[r]:
                            ww = min(WIN, cw - w * WIN)
                            for t in range(tn):
                                s = tb + t
                                m = msgpool.tile([P, H], bf16, tag="msg")
                                nc.gpsimd.indirect_dma_start(
                                    out=m[:], out_offset=None,
                                    in_=h_full[l][:],
                                    in_offset=bass.IndirectOffsetOnAxis(
                                        ap=idx_t[:, s:s + 1], axis=0))
                                hot = hotpool.tile([P, WIN], bf16, tag="hot")
                                nc.vector.tensor_scalar(
                                    out=hot[:, :ww], in0=iota_bf[:, :ww],
                                    scalar1=loc_t[:, s:s + 1],
                                    scalar2=alp_t[:, s:s + 1],
                                    op0=OP.is_equal, op1=OP.mult)
                                nc.tensor.matmul(
                                    out=psa[:, w * WIN:w * WIN + ww],
                                    lhsT=m[:], rhs=hot[:, :ww],
                                    start=(t == 0), stop=(t == tn - 1))
                        if r % 2 == 0:
                            nc.vector.tensor_copy(mean[:, r, :cw], psa[:, :cw])
                        else:
                            nc.scalar.copy(mean[:, r, :cw], psa[:, :cw])

                    psb = psB.tile([P, CHUNK], f32, tag="psB")
                    # split transform at the 512-col PSUM bank boundary
                    for h0 in range(0, cw, 512):
                        hw_ = min(512, cw - h0)
                        nc.tensor.matmul(
                            out=psb[:, h0:h0 + hw_], lhsT=relw_t[:, root_i, :],
                            rhs=hT[:, c * CHUNK + h0:c * CHUNK + h0 + hw_],
                            start=True, stop=False)
                        for r in range(R):
                            nc.tensor.matmul(out=psb[:, h0:h0 + hw_],
                                             lhsT=relw_t[:, l * R + r, :],
                                             rhs=mean[:, r, h0:h0 + hw_],
                                             start=False, stop=(r == R - 1))

                    nc.vector.tensor_scalar(
                        out=outb[:, c * CHUNK:c * CHUNK + cw], in0=psb[:, :cw],
                        scalar1=1.0, scalar2=None, op0=OP.mult, op1=OP.add,
                        accum_out=sum_parts[:, c:c + 1])
                    nc.scalar.activation(sq_scr[:, :cw], psb[:, :cw], AF.Square,
                                         accum_out=sq_parts[:, c:c + 1])

                # ---------- BatchNorm + ReLU ----------
                st = workpool.tile([H, 2], f32, tag="stats")
                nc.vector.reduce_sum(st[:, 0:1], sum_parts[:],
                                     axis=mybir.AxisListType.X)
                nc.vector.reduce_sum(st[:, 1:2], sq_parts[:],
                                     axis=mybir.AxisListType.X)
                nc.sync.dma_start(stats_in[:], st[:])
                nc.gpsimd.collective_compute(
                    "AllReduce", OP.add, replica_groups=[cores],
                    ins=[stats_in[:]], outs=[stats_out[:]])
                stg = workpool.tile([H, 8], f32, tag="stg")
                nc.sync.dma_start(stg[:, 0:2], stats_out[:])
                nc.vector.tensor_scalar(out=stg[:, 2:3], in0=stg[:, 0:1],
                                        scalar1=1.0 / N, scalar2=None,
                                        op0=OP.mult)
                nc.vector.tensor_scalar(out=stg[:, 3:4], in0=stg[:, 1:2],
                                        scalar1=1.0 / N, scalar2=None,
                                        op0=OP.mult)
                nc.vector.tensor_tensor(out=stg[:, 4:5], in0=stg[:, 2:3],
                                        in1=stg[:, 2:3], op=OP.mult)
                nc.vector.tensor_tensor(out=stg[:, 4:5], in0=stg[:, 3:4],
                                        in1=stg[:, 4:5], op=OP.subtract)
                nc.vector.tensor_scalar(out=stg[:, 4:5], in0=stg[:, 4:5],
                                        scalar1=cfg.EPS, scalar2=None,
                                        op0=OP.add)
                nc.scalar.sqrt(stg[:, 5:6], stg[:, 4:5])
                nc.vector.reciprocal(stg[:, 6:7], stg[:, 5:6])
                nc.vector.tensor_tensor(out=stg[:, 6:7], in0=stg[:, 6:7],
                                        in1=bng_t[:, l:l + 1], op=OP.mult)
                nc.vector.tensor_tensor(out=stg[:, 7:8], in0=stg[:, 6:7],
                                        in1=stg[:, 2:3], op=OP.mult)
                nc.vector.tensor_tensor(out=stg[:, 7:8], in0=bnb_t[:, l:l + 1],
                                        in1=stg[:, 7:8], op=OP.subtract)
                if cfg.DEBUG:
                    nc.sync.dma_start(dbg_outb[l], outb[:])
                    nc.sync.dma_start(dbg_stg[l], stg[:])
                nc.scalar.activation(hT[:], outb[:], AF.Relu,
                                     bias=stg[:, 7:8], scale=stg[:, 6:7])

                if l + 1 < L:
                    emit_transpose_store(l + 1)

            # ---------- global mean pool + output MLP ----------
            psp = psB.tile([G, CHUNK], f32, tag="psB")
            for b in range(nblk):
                bw = min(P, NS - b * P)
                pst = psT.tile([P, P], bf16, tag="psT")
                nc.tensor.transpose(pst[:bw, :P], hT[:, b * P:b * P + bw],
                                    ident[:])
                rm = workpool.tile([P, P], bf16, tag="rm")
                nc.vector.tensor_copy(rm[:bw, :], pst[:bw, :P])
                ind = hotpool.tile([P, G], bf16, tag="ind")
                nc.vector.tensor_scalar(out=ind[:bw, :], in0=iota_bf[:bw, :G],
                                        scalar1=gids_t[:bw, b:b + 1],
                                        scalar2=None, op0=OP.is_equal)
                nc.tensor.matmul(out=psp[:, :H], lhsT=ind[:bw, :],
                                 rhs=rm[:bw, :], start=(b == 0),
                                 stop=(b == nblk - 1))
            poolt = workpool.tile([G, H], f32, tag="poolt")
            nc.vector.tensor_copy(poolt[:], psp[:, :H])
            nc.sync.dma_start(pool_in[:], poolt[:])
            nc.gpsimd.collective_compute(
                "AllReduce", OP.add, replica_groups=[cores],
                ins=[pool_in[:]], outs=[pool_out[:]])
            poolg = workpool.tile([G, H], f32, tag="poolg")
            nc.sync.dma_start(poolg[:], pool_out[:])
            if cfg.DEBUG:
                nc.gpsimd.dma_start(dbg_pool[:], pool_out[:])

            pstT = psT.tile([P, G], f32, tag="psTf")
            nc.tensor.transpose(pstT[:, :G], poolg[:], identf[:G, :G])
            poolT = workpool.tile([P, G], f32, tag="poolT")
            nc.vector.tensor_copy(poolT[:], pstT[:, :G])

            psl = psB.tile([C, CHUNK], f32, tag="psB")
            nc.tensor.matmul(out=psl[:, :G], lhsT=w_out_t[:], rhs=poolT[:],
                             start=True, stop=True)
            logit = workpool.tile([C, G], f32, tag="logit")
            nc.vector.tensor_tensor(out=logit[:], in0=psl[:, :G], in1=invg_t[:],
                                    op=OP.mult)
            logit2 = workpool.tile([C, G], f32, tag="logit2")
            nc.scalar.activation(logit2[:], logit[:], AF.Sigmoid,
                                 bias=b_out_t[:, 0:1], scale=1.0)
            nc.sync.dma_start(out_d[:], logit2[:])

    return nc


def _make_in_maps(cfg, plan, inputs):
    H, C, G, F, NS, R, L = cfg.H, cfg.C, cfg.G, cfg.F, cfg.NS, cfg.R, cfg.L
    x = np.asarray(inputs["x"], np.float32)
    batch = np.asarray(inputs["batch"])

    np_dt = BF16 if cfg.DT == "bf16" else np.float32
    relw = np.empty((L * R + L, P, H), np_dt)
    rel_w = np.asarray(inputs["rel_w"], np.float32)
    root_w = np.asarray(inputs["root_w"], np.float32)
    for l in range(L):
        for r in range(R):
            relw[l * R + r] = rel_w[l, r].astype(np_dt)
        relw[L * R + l] = root_w[l].astype(np_dt)

    bng = np.ascontiguousarray(np.asarray(inputs["bn_g"], np.float32).T)
    bnb = np.ascontiguousarray(np.asarray(inputs["bn_b"], np.float32).T)
    b_in = np.asarray(inputs["b_in"], np.float32).reshape(H, 1)
    b_out = np.asarray(inputs["b_out"], np.float32).reshape(C, 1)
    w_in = np.asarray(inputs["w_in"], np.float32)
    w_out = np.asarray(inputs["w_out"], np.float32)
    invg = np.ascontiguousarray(
        np.broadcast_to(plan["inv_gcnt"][None, :], (C, G)))
    relw_p = np.ascontiguousarray(relw.transpose(1, 0, 2))  # [P, 42, H]
    iota_np = np.broadcast_to(np.arange(cfg.WIN, dtype=np.float32)[None, :],
                              (P, cfg.WIN)).astype(BF16).copy()
    ident_np = np.eye(P, dtype=np.float32).astype(BF16)
    identf_np = np.eye(P, dtype=np.float32)

    nblk = math.ceil(NS / P)
    in_maps = []
    for c in range(cfg.NC):
        lo, hi = c * NS, (c + 1) * NS
        xT = np.ascontiguousarray(x[lo:hi].T)
        gids = np.full((P, nblk), -1.0, np.float32)
        bseg = batch[lo:hi].astype(np.float32)
        for b in range(nblk):
            bw = min(P, NS - b * P)
            gids[:bw, b] = bseg[b * P:b * P + bw]
        in_maps.append(dict(
            xT=xT, idxA=np.ascontiguousarray(plan["idxA"][c]),
            iotain=iota_np, identb=ident_np, identf=identf_np,
            locA=np.ascontiguousarray(plan["locA"][c]),
            alphaA=np.ascontiguousarray(plan["alphaA"][c]),
            w_in=w_in, b_in=b_in, relw=relw_p, bng=bng, bnb=bnb,
            w_out=w_out, b_out=b_out, gids=gids, invg=invg,
        ))
    return in_maps


def _plan_key(cfg, plan):
    def _freeze(x):
        if isinstance(x, (list, tuple)):
            return tuple(_freeze(v) for v in x)
        return x
    return (cfg.N, cfg.E, cfg.F, cfg.H, cfg.R, cfg.G, cfg.C, cfg.L, cfg.NC,
            cfg.CHUNK, cfg.WIN, cfg.DT, cfg.DEBUG, plan["S_total"],
            _freeze(plan["chunk_tiles"]))


class _Runner:
    """Compile-once PJRT runner (mirrors run_bass_kernel_spmd's axon path,
    but keeps the jitted callable so repeat invocations skip retrace,
    XLA recompile, and executable reload)."""

    def __init__(self, cfg, plan):
        import jax
        from jax.sharding import Mesh, PartitionSpec, NamedSharding
        try:
            from jax.experimental.shard_map import shard_map
        except ImportError:
            from jax import shard_map
        from concourse.bass2jax import (
            _bass_exec_p, partition_id_tensor, install_neuronx_cc_hook)

        self.cfg = cfg
        nc = _build_nc(cfg, plan)
        if not nc.is_finalized():
            nc.finalize()
        self.nc = nc
        install_neuronx_cc_hook()
        assert nc.dbg_addr is None

        pname = nc.partition_id_tensor.name if nc.partition_id_tensor else None
        in_names, out_names, out_avals, self.zero_shapes = [], [], [], []
        for alloc in nc.m.functions[0].allocations:
            if not isinstance(alloc, mybir.MemoryLocationSet):
                continue
            name = alloc.memorylocations[0].name
            if alloc.kind == "ExternalInput":
                if name != pname:
                    in_names.append(name)
            elif alloc.kind == "ExternalOutput":
                out_names.append(name)
                shape = tuple(alloc.tensor_shape)
                dtype = mybir.dt.np(alloc.dtype)
                out_avals.append(jax.core.ShapedArray(shape, dtype))
                self.zero_shapes.append((shape, dtype))
        self.in_names, self.out_names = in_names, out_names
        self.out_avals = out_avals
        n_params, n_outs = len(in_names), len(out_avals)
        all_names = in_names + out_names + ([pname] if pname else [])
        donate = tuple(range(n_params, n_params + n_outs))

        def _body(*args):
            operands = list(args)
            if pname is not None:
                operands.append(partition_id_tensor())
            return tuple(_bass_exec_p.bind(
                *operands, out_avals=tuple(out_avals),
                in_names=tuple(all_names), out_names=tuple(out_names),
                lowering_input_output_aliases=(),
                sim_require_finite=True, sim_require_nnan=True, nc=nc))

        devices = jax.devices()[:cfg.NC]
        assert len(devices) == cfg.NC
        self.mesh = Mesh(np.asarray(devices), ("core",))
        self.shard = NamedSharding(self.mesh, PartitionSpec("core"))
        self.jit = jax.jit(
            shard_map(_body, mesh=self.mesh,
                      in_specs=(PartitionSpec("core"),) * (n_params + n_outs),
                      out_specs=(PartitionSpec("core"),) * n_outs,
                      check_rep=False),
            donate_argnums=donate, keep_unused=True)

    def concat_inputs(self, in_maps):
        return [np.concatenate([np.asarray(m[n]) for m in in_maps], axis=0)
                for n in self.in_names]

    def put_inputs(self, concat):
        import jax
        dev = [jax.device_put(a, self.shard) for a in concat]
        jax.block_until_ready(dev)
        return dev

    def zeros(self, device=False):
        import jax
        zs = [np.zeros((self.cfg.NC * s[0], *s[1:]), dt)
              for (s, dt) in self.zero_shapes]
        if device:
            zs = [jax.device_put(z, self.shard) for z in zs]
            jax.block_until_ready(zs)
        return zs

    def __call__(self, ins, zeros):
        return self.jit(*ins, *zeros)

    def out_core0(self, outs):
        i = self.out_names.index("out")
        shape = self.out_avals[i].shape
        return np.asarray(outs[i]).reshape(self.cfg.NC, *shape)[0]


_RUNNER_CACHE = {}


def _get_runner(cfg, plan):
    key = _plan_key(cfg, plan)
    r = _RUNNER_CACHE.get(key)
    if r is None:
        r = _Runner(cfg, plan)
        _RUNNER_CACHE[key] = r
    return r


def _run(cfg, inputs, **kw):
    plan = _plan(cfg, np.asarray(inputs["edge_index"]),
                 np.asarray(inputs["edge_type"]), np.asarray(inputs["batch"]))
    runner = _get_runner(cfg, plan)
    in_maps = _make_in_maps(cfg, plan, inputs)
    concat = runner.concat_inputs(in_maps)
    outs = runner(concat, runner.zeros())
    out = runner.out_core0(outs)
    return np.ascontiguousarray(np.asarray(out).T.astype(np.float32)), runner


def kernel(**inputs):
    cfg = Cfg()
    out, _ = _run(cfg, inputs)
    return out

